# revision 1
# baseline (speedup 1.0000x reference)
"""GCN encoder (edge-wise message passing) on 8 Trainium2 NeuronCores.

Strategy (dst-range sharding):
  - Host: sort edges by dst, shard by dst-range (core r owns nodes
    [r*NLOC, (r+1)*NLOC)), group edges into 128-node windows, pad each
    (window, src-half) group to 128-multiples. Degree / index prep on host.
  - Device: BN stats via ACT-accumulate + tiny AllReduce, folded into W_i.
    Pre-pass computes f_e (edge- and feature-major) and the loop-invariant
    per-edge base = f_e @ Wh_mid + p*w_p, storing base to HBM (bf16), and
    performs the iter-0 scatter (f_n) directly from f_e.
    Each iteration: gather g_s[src], g_d[dst] rows via dma_gather from
    node tables, eh = relu(base + g_s + g_d), scatter-mean via one-hot
    matmul into PSUM per 128-node window, then build next tables
    g_s|g_d = h @ [Wh_src|Wh_dst] (+b_h) and AllGather them.
"""
import sys
sys.path.insert(0, "/opt/trn_rl_repo")

import os
import numpy as np
import ml_dtypes
from contextlib import ExitStack

from concourse import bass, bacc, mybir, tile, masks
from concourse.bass_utils import run_bass_kernel_spmd

f32 = mybir.dt.float32
bf16 = mybir.dt.bfloat16
i16 = mybir.dt.int16
i32 = mybir.dt.int32
AO = mybir.AluOpType
AF = mybir.ActivationFunctionType

NCORES = 8
DEPTH = 3
EPS = 1e-5
GW = 4            # windows per superwindow
STAT_SLICE = 2048

bfl = ml_dtypes.bfloat16


def _ru(x, m):
    return (x + m - 1) // m * m


class Plan:
    """Host-side preprocessing: sharding, sorting, padding, index layout."""

    def __init__(self, src, dst, N):
        E = src.shape[0]
        self.N, self.E = N, E
        self.NLOC = (N + NCORES - 1) // NCORES
        self.NWIN = (self.NLOC + 127) // 128
        self.NLOCP = self.NWIN * 128
        self.NGLOB = NCORES * self.NLOCP
        # src-half split: largest rank-multiple of NLOCP that fits int16
        self.SPLIT = min((32768 // self.NLOCP) * self.NLOCP, self.NGLOB)
        assert self.NGLOB - self.SPLIT < 32768

        owner = dst // self.NLOC
        local = dst - owner * self.NLOC
        win = local >> 7
        self.ohval_all = (local & 127).astype(np.float32)
        srcrow = (src // self.NLOC) * self.NLOCP + (src % self.NLOC)
        half = (srcrow >= self.SPLIT).astype(np.int64)
        self.srcrow, self.local, self.owner, self.win, self.half = (
            srcrow, local, owner, win, half)

        key = (owner * self.NWIN + win) * 2 + half
        self.order = np.argsort(key, kind="stable")
        cnt = np.bincount(key, minlength=NCORES * self.NWIN * 2)
        cnt = cnt.reshape(NCORES, self.NWIN, 2)
        self.capA = np.maximum(_ru(cnt[:, :, 0].max(0), 128), 128)
        self.capB = _ru(cnt[:, :, 1].max(0), 128)
        self.cnt = cnt

        # superwindows
        self.NSW = (self.NWIN + GW - 1) // GW
        self.sw_windows = [list(range(s * GW, min((s + 1) * GW, self.NWIN)))
                           for s in range(self.NSW)]
        # slot layout: per sw, [A_w0..A_wk | B_w0..B_wk]
        self.slotA = np.zeros(self.NWIN, np.int64)   # slot offset of A group
        self.slotB = np.zeros(self.NWIN, np.int64)
        self.sw_off = np.zeros(self.NSW + 1, np.int64)
        off = 0
        for s, ws in enumerate(self.sw_windows):
            self.sw_off[s] = off
            a = off
            for w in ws:
                self.slotA[w] = a
                a += self.capA[w]
            for w in ws:
                self.slotB[w] = a
                a += self.capB[w]
            off = a
        self.sw_off[self.NSW] = off
        self.ES = int(off)
        self.sw_capA = [int(sum(self.capA[w] for w in ws))
                        for ws in self.sw_windows]
        self.sw_capB = [int(sum(self.capB[w] for w in ws))
                        for ws in self.sw_windows]
        self.sw_cap = [a + b for a, b in zip(self.sw_capA, self.sw_capB)]
        self.EMAX4 = _ru(max(int((owner == r).sum()) for r in range(NCORES)), 512)
        self.Q4 = self.EMAX4 // 4

    def signature(self):
        return (self.N, self.E, tuple(self.capA), tuple(self.capB))


def _host_inputs(plan, e, p, src, dst):
    """Build the per-core input arrays."""
    NLOC, NWIN, ES = plan.NLOC, plan.NWIN, plan.ES
    order, cnt = plan.order, plan.cnt
    deg = np.maximum(np.bincount(dst, minlength=plan.N), 1).astype(np.float32)
    invd = 1.0 / deg

    in_maps = []
    pos = 0
    # order slices per (r, w, h) in key order
    slices = {}
    for r in range(NCORES):
        for w in range(NWIN):
            for h in range(2):
                c = int(cnt[r, w, h])
                slices[(r, w, h)] = order[pos:pos + c]
                pos += c
    assert pos == plan.E

    for r in range(NCORES):
        efm = np.zeros((34, ES), np.float32)
        efm[32, :] = 1.0
        gsx = np.zeros(ES, np.int16)
        gdx = np.zeros(ES, np.int16)
        ohv = np.full(ES, -5.0, np.float32)
        for w in range(NWIN):
            for h, base_slot in ((0, plan.slotA[w]), (1, plan.slotB[w])):
                idx = slices[(r, w, h)]
                n = idx.shape[0]
                sl = slice(base_slot, base_slot + n)
                efm[0:32, sl] = e[idx].T
                efm[33, sl] = p[idx, 0]
                gsx[sl] = plan.srcrow[idx] - (plan.SPLIT if h else 0)
                gdx[sl] = plan.local[idx]
                ohv[sl] = plan.ohval_all[idx]

        # wrap idxs per superwindow: [16, cap/16] replicated x8
        def wrap(arr):
            out = np.zeros((128, ES // 16), np.int16)
            for s in range(plan.NSW):
                o0, o1 = int(plan.sw_off[s]), int(plan.sw_off[s + 1])
                seg = arr[o0:o1].reshape(-1, 16).T
                out[:, o0 // 16:o1 // 16] = np.tile(seg, (8, 1))
            return out

        soh = ohv.reshape(-1, 128).T.copy()  # [128, ES//128]
        ivl = np.ones(plan.NLOCP, np.float32)
        lo, hi = r * NLOC, min((r + 1) * NLOC, plan.N)
        ivl[:hi - lo] = invd[lo:hi]
        invdeg = ivl.reshape(NWIN, 128).T.copy()  # [128, NWIN]

        mask = plan.owner == np.int64(r)
        er = e[mask]
        epad = np.zeros((plan.EMAX4, 32), np.float32)
        epad[:er.shape[0]] = er
        e4 = epad.reshape(4, plan.Q4, 32).transpose(0, 2, 1).reshape(128, plan.Q4)

        in_maps.append({
            "efm": efm.astype(bfl),
            "gs_idx": wrap(gsx),
            "gd_idx": wrap(gdx),
            "soh": soh,
            "invdeg": invdeg,
            "e4": e4.astype(bfl),
        })
    return in_maps


def _weight_inputs(plan, gamma, beta, W_i, b_i, W_h, b_h):
    OUT = W_i.shape[1]
    whmid = np.zeros((OUT, 128), np.float32)
    whmid[:, :OUT] = W_h[OUT:2 * OUT]
    wp2 = np.zeros((2, 128), np.float32)
    wp2[1, :OUT] = W_h[2 * OUT]
    whsd = np.zeros((OUT + 1, 256), np.float32)
    whsd[:OUT, 0:OUT] = W_h[0:OUT]
    whsd[:OUT, 128:128 + OUT] = W_h[2 * OUT + 1:3 * OUT + 1]
    whsd[OUT, 128:128 + OUT] = b_h
    return {
        "W_i": W_i.astype(np.float32),
        "b_i": b_i.reshape(OUT, 1).astype(np.float32),
        "gamma": gamma.reshape(32, 1).astype(np.float32),
        "beta": beta.reshape(32, 1).astype(np.float32),
        "whmid": whmid.astype(bfl),
        "wp2": wp2.astype(bfl),
        "whsd": whsd.astype(bfl),
    }


def _build(plan, OUT):
    """Build + compile the SPMD Bass program for this plan."""
    NWIN, NSW, ES = plan.NWIN, plan.NSW, plan.ES
    NLOCP, NGLOB, SPLIT = plan.NLOCP, plan.NGLOB, plan.SPLIT
    IN = 32

    nc = bacc.Bacc("TRN2", target_bir_lowering=False, debug=False,
                   num_devices=NCORES)

    efm = nc.dram_tensor("efm", [34, ES], bf16, kind="ExternalInput")
    gs_idx = nc.dram_tensor("gs_idx", [128, ES // 16], i16, kind="ExternalInput")
    gd_idx = nc.dram_tensor("gd_idx", [128, ES // 16], i16, kind="ExternalInput")
    soh = nc.dram_tensor("soh", [128, ES // 128], f32, kind="ExternalInput")
    invdeg = nc.dram_tensor("invdeg", [128, NWIN], f32, kind="ExternalInput")
    e4 = nc.dram_tensor("e4", [128, plan.Q4], bf16, kind="ExternalInput")
    W_i = nc.dram_tensor("W_i", [IN, OUT], f32, kind="ExternalInput")
    b_i = nc.dram_tensor("b_i", [OUT, 1], f32, kind="ExternalInput")
    gamma = nc.dram_tensor("gamma", [IN, 1], f32, kind="ExternalInput")
    beta = nc.dram_tensor("beta", [IN, 1], f32, kind="ExternalInput")
    whmid = nc.dram_tensor("whmid", [OUT, 128], bf16, kind="ExternalInput")
    wp2 = nc.dram_tensor("wp2", [2, 128], bf16, kind="ExternalInput")
    whsd = nc.dram_tensor("whsd", [OUT + 1, 256], bf16, kind="ExternalInput")

    out_fn = nc.dram_tensor("out_fn", [NLOCP, OUT], f32, kind="ExternalOutput")
    out_h = nc.dram_tensor("out_h", [NLOCP, OUT], f32, kind="ExternalOutput")

    inv_E = 1.0 / plan.E

    with tile.TileContext(nc) as tc:
        with ExitStack() as ctx:
            cpool = ctx.enter_context(tc.tile_pool(name="cpool", bufs=1))
            pool = ctx.enter_context(tc.tile_pool(name="pool", bufs=2))
            spool = ctx.enter_context(tc.tile_pool(name="spool", bufs=2))
            psum = ctx.enter_context(tc.tile_pool(name="psum", bufs=2,
                                                  space="PSUM"))
            # p2-tagged tiles rotate 4 deep (mm2 + mm1e pipelines)
            dram = ctx.enter_context(tc.tile_pool(name="dram", bufs=1,
                                                  space="DRAM"))

            # ---- constants ----
            iota_i = cpool.tile([128, 128], i32)
            nc.gpsimd.iota(iota_i[:], pattern=[[1, 128]], base=0,
                           channel_multiplier=0)
            iota_b = cpool.tile([128, 128], bf16)
            nc.vector.tensor_copy(iota_b[:], iota_i[:])
            ident = cpool.tile([128, 128], f32)
            masks.make_identity(nc, ident[:])

            whmid_t = cpool.tile([OUT, 128], bf16)
            nc.sync.dma_start(whmid_t[:], whmid[:])
            wp2_t = cpool.tile([34, 128], bf16)
            nc.sync.dma_start(wp2_t[32:34, :], wp2[:])
            whsd_t = cpool.tile([OUT + 1, 256], bf16)
            nc.sync.dma_start(whsd_t[:], whsd[:])
            invdeg_t = cpool.tile([128, NWIN], f32)
            nc.sync.dma_start(invdeg_t[:], invdeg[:])

            # ---- BN stats: per-core partial sums of e, e^2 ----
            nsl = (plan.Q4 + STAT_SLICE - 1) // STAT_SLICE
            parts = cpool.tile([128, 2 * nsl], f32)
            for s in range(nsl):
                c0, c1 = s * STAT_SLICE, min((s + 1) * STAT_SLICE, plan.Q4)
                esl = spool.tile([128, STAT_SLICE], bf16, tag="esl")
                nc.sync.dma_start(esl[:, :c1 - c0], e4[:, c0:c1])
                junk = spool.tile([128, STAT_SLICE], f32, tag="junk")
                nc.scalar.activation(junk[:, :c1 - c0], esl[:, :c1 - c0],
                                     AF.Copy, accum_out=parts[:, s:s + 1])
                nc.scalar.activation(junk[:, :c1 - c0], esl[:, :c1 - c0],
                                     AF.Square,
                                     accum_out=parts[:, nsl + s:nsl + s + 1])
            sums = cpool.tile([128, 2], f32)
            junk2 = cpool.tile([128, nsl], f32)
            nc.scalar.activation(junk2[:], parts[:, 0:nsl], AF.Copy,
                                 accum_out=sums[:, 0:1])
            nc.scalar.activation(junk2[:], parts[:, nsl:2 * nsl], AF.Copy,
                                 accum_out=sums[:, 1:2])
            ar_in = dram.tile([128, 2], f32)
            ar_out = dram.tile([128, 2], f32)
            nc.sync.dma_start(ar_in[:], sums[:])
            nc.gpsimd.collective_compute(
                "AllReduce", AO.add, replica_groups=[list(range(NCORES))],
                ins=[ar_in.opt()], outs=[ar_out.opt()])
            g4 = cpool.tile([32, 4, 2], f32)
            nc.sync.dma_start(
                g4[:], ar_out[:].rearrange("(g p) k -> p g k", g=4))
            t1 = cpool.tile([32, 2], f32)
            t2 = cpool.tile([32, 2], f32)
            tot = cpool.tile([32, 2], f32)
            nc.vector.tensor_tensor(t1[:], g4[:, 0, :], g4[:, 1, :], AO.add)
            nc.vector.tensor_tensor(t2[:], g4[:, 2, :], g4[:, 3, :], AO.add)
            nc.vector.tensor_tensor(tot[:], t1[:], t2[:], AO.add)
            mu = cpool.tile([32, 1], f32)
            nc.vector.tensor_scalar(mu[:], tot[:, 0:1], inv_E, None, op0=AO.mult)
            ms = cpool.tile([32, 1], f32)
            nc.vector.tensor_scalar(ms[:], tot[:, 1:2], inv_E, None, op0=AO.mult)
            var = cpool.tile([32, 1], f32)
            mu2 = cpool.tile([32, 1], f32)
            nc.vector.tensor_tensor(mu2[:], mu[:], mu[:], AO.mult)
            nc.vector.tensor_tensor(var[:], ms[:], mu2[:], AO.subtract)
            epsb = cpool.tile([32, 1], f32)
            nc.vector.memset(epsb[:], EPS)
            std = cpool.tile([32, 1], f32)
            nc.scalar.activation(std[:], var[:], AF.Sqrt, bias=epsb[:])
            rstd = cpool.tile([32, 1], f32)
            nc.vector.reciprocal(rstd[:], std[:])
            gam_t = cpool.tile([32, 1], f32)
            nc.sync.dma_start(gam_t[:], gamma[:])
            bet_t = cpool.tile([32, 1], f32)
            nc.sync.dma_start(bet_t[:], beta[:])
            a_t = cpool.tile([32, 1], f32)
            nc.vector.tensor_tensor(a_t[:], gam_t[:], rstd[:], AO.mult)
            nma = cpool.tile([32, 1], f32)
            nc.vector.scalar_tensor_tensor(nma[:], mu[:], -1.0, a_t[:],
                                           op0=AO.mult, op1=AO.mult)
            c_t = cpool.tile([32, 1], f32)
            nc.vector.tensor_tensor(c_t[:], bet_t[:], nma[:], AO.add)

            wi_t = cpool.tile([32, OUT], f32)
            nc.sync.dma_start(wi_t[:], W_i[:])
            wif = cpool.tile([32, OUT], f32)
            nc.vector.tensor_scalar(wif[:], wi_t[:], a_t[:], None, op0=AO.mult)
            bi_t = cpool.tile([OUT, 1], f32)
            nc.sync.dma_start(bi_t[:], b_i[:])
            pb = psum.tile([OUT, 1], f32, tag="p2", bufs=4)
            nc.tensor.matmul(pb[:], wif[:], c_t[:], start=True, stop=True)
            bcol = cpool.tile([OUT, 1], f32)
            nc.vector.tensor_tensor(bcol[:], pb[:], bi_t[:], AO.add)
            scr = dram.tile([OUT, 1], f32)
            nc.sync.dma_start(scr[:], bcol[:])
            wiaug = cpool.tile([33, OUT], bf16)
            nc.vector.tensor_copy(wiaug[0:32, :], wif[:])
            nc.gpsimd.dma_start(wiaug[32:33, :],
                                scr[:].rearrange("a b -> b a"))

            # ---- DRAM intermediates ----
            baseH = dram.tile([ES, 128], bf16)
            baseHv = baseH[:].rearrange("(b p) c -> p b c", p=128)
            tloc = [dram.tile([NLOCP, 256], bf16, name=f"tloc{k}",
                              tag=f"tloc{k}") for k in range(DEPTH)]
            tglob = [dram.tile([NGLOB, 256], bf16, name=f"tglob{k}",
                               tag=f"tglob{k}") for k in range(DEPTH)]

            def finalize_window(it, w, pw, wl):
                """pw[:, wl*128:(wl+1)*128] holds the scatter sums of window w."""
                pwv = pw[:, wl * 128:wl * 128 + OUT]
                h_t = pool.tile([128, 128], f32, tag="h_t")
                nc.vector.tensor_scalar(h_t[:, :OUT], pwv,
                                        invdeg_t[:, w:w + 1], None, op0=AO.mult)
                if it == 0:
                    nc.sync.dma_start(out_fn[w * 128:(w + 1) * 128, :],
                                      h_t[:, :OUT])
                if it == DEPTH:
                    nc.sync.dma_start(out_h[w * 128:(w + 1) * 128, :],
                                      h_t[:, :OUT])
                    return
                pt = psum.tile([128, 128], f32, tag="p2", bufs=4)
                nc.tensor.transpose(pt[:OUT, :], h_t[:, :OUT], ident[:])
                hT = pool.tile([OUT + 1, 128], bf16, tag="hT")
                nc.vector.memset(hT[:], 1.0)
                nc.vector.tensor_copy(hT[:OUT, :], pt[:OUT, :])
                ptab = psum.tile([128, 512], f32, tag="p1")
                nc.tensor.matmul(ptab[:, :256], hT[:], whsd_t[:],
                                 start=True, stop=True)
                ttab = pool.tile([128, 256], bf16, tag="ttab")
                nc.scalar.copy(ttab[:], ptab[:, :256])
                nc.sync.dma_start(tloc[it][w * 128:(w + 1) * 128, :], ttab[:])

            def window_chunks(s, w):
                """(block, ohcol) pairs of window w within superwindow s."""
                o = int(plan.sw_off[s])
                res = []
                for base_slot, cap in ((plan.slotA[w], plan.capA[w]),
                                       (plan.slotB[w], plan.capB[w])):
                    b0 = (int(base_slot) - o) // 128
                    g0 = int(base_slot) // 128
                    for c in range(int(cap) // 128):
                        res.append((b0 + c, g0 + c))
                return res

            PHASE = int(os.environ.get("GNN_PHASE", "3"))
            # ---- pre-pass + iter 0 ----
            for s in range(NSW) if PHASE >= 2 else []:
                cap = plan.sw_cap[s]
                nblk = cap // 128
                o0 = int(plan.sw_off[s])
                efm_t = pool.tile([34, cap], bf16, tag="big0")
                nc.sync.dma_start(efm_t[:], efm[:, o0:o0 + cap])
                sohc = pool.tile([128, nblk], f32, tag="sohc")
                nc.sync.dma_start(sohc[:], soh[:, o0 // 128:o0 // 128 + nblk])
                feT = pool.tile([OUT, cap], bf16, tag="big1")
                baseC = pool.tile([128, nblk, 128], bf16, tag="big2")

                for g0 in range(0, cap, 512):
                    g1 = min(g0 + 512, cap)
                    p1 = psum.tile([OUT, 512], f32, tag="p1")
                    nc.tensor.matmul(p1[:, :g1 - g0], wiaug[:],
                                     efm_t[0:33, g0:g1], start=True, stop=True)
                    nc.scalar.activation(feT[:, g0:g1], p1[:, :g1 - g0], AF.Relu)

                pw_tile = psum.tile([128, 512], f32, tag="pw")

                for wl, w in enumerate(plan.sw_windows[s]):
                    chunks = window_chunks(s, w)
                    for ci, (b, gb) in enumerate(chunks):
                        sl = slice(b * 128, (b + 1) * 128)
                        p2 = psum.tile([128, 128], f32, tag="p2", bufs=4)
                        nc.tensor.matmul(p2[:], feT[:, sl], whmid_t[:],
                                         start=True, stop=False)
                        nc.tensor.matmul(p2[:], efm_t[32:34, sl],
                                         wp2_t[32:34, :],
                                         start=False, stop=True)
                        nc.vector.tensor_copy(baseC[:, b, :], p2[:])
                        p3 = psum.tile([128, 128], f32, tag="p2", bufs=4)
                        nc.tensor.matmul(p3[:, :OUT], efm_t[0:33, sl],
                                         wiaug[:], start=True, stop=True)
                        fee = pool.tile([128, 128], bf16, tag="fee")
                        nc.scalar.activation(fee[:, :OUT], p3[:, :OUT], AF.Relu)
                        S = pool.tile([128, 128], bf16, tag="S", bufs=4)
                        nc.vector.tensor_scalar(S[:], iota_b[:],
                                                sohc[:, b:b + 1], None,
                                                op0=AO.is_equal)
                        nc.tensor.matmul(
                            pw_tile[:, wl * 128:wl * 128 + OUT], S[:],
                            fee[:, :OUT], start=(ci == 0),
                            stop=(ci == len(chunks) - 1))
                nc.sync.dma_start(baseHv[:, o0 // 128:o0 // 128 + nblk, :],
                                  baseC[:])
                for wl, w in enumerate(plan.sw_windows[s]):
                    finalize_window(0, w, pw_tile, wl)

            if PHASE >= 2:
                nc.gpsimd.collective_compute(
                    "AllGather", AO.bypass,
                    replica_groups=[list(range(NCORES))],
                    ins=[tloc[0].opt()], outs=[tglob[0].opt()])

            # ---- iterations 1..DEPTH ----
            for it in range(1, DEPTH + 1) if PHASE >= 3 else []:
                tg_v = tglob[it - 1][:].rearrange("n (h c) -> n h c", h=2)
                tl_v = tloc[it - 1][:].rearrange("n (h c) -> n h c", h=2)
                for s in range(NSW):
                    cap = plan.sw_cap[s]
                    nblk = cap // 128
                    capA = plan.sw_capA[s]
                    o0 = int(plan.sw_off[s])
                    GS = pool.tile([128, nblk, 128], bf16, tag="big0")
                    GD = pool.tile([128, nblk, 128], bf16, tag="big1")
                    BASE = pool.tile([128, nblk, 128], bf16, tag="big2")
                    nc.sync.dma_start(BASE[:],
                                      baseHv[:, o0 // 128:o0 // 128 + nblk, :])
                    sohc = pool.tile([128, nblk], f32, tag="sohc")
                    nc.sync.dma_start(sohc[:],
                                      soh[:, o0 // 128:o0 // 128 + nblk])
                    gsix = pool.tile([128, cap // 16], i16, tag="gsix")
                    nc.sync.dma_start(gsix[:], gs_idx[:, o0 // 16:o0 // 16 + cap // 16])
                    gdix = pool.tile([128, cap // 16], i16, tag="gdix")
                    nc.sync.dma_start(gdix[:], gd_idx[:, o0 // 16:o0 // 16 + cap // 16])

                    def gather_piece(dst_tile, src_view, ix_tile, off, m):
                        b0 = off // 128
                        nc.gpsimd.dma_gather(
                            dst_tile[:, b0:b0 + m // 128, :], src_view,
                            ix_tile[:16, off // 16:(off + m) // 16],
                            m, m, 128, elem_step=256)

                    # pieces of <=1024 slots, within A then B regions
                    pieces = []
                    for r0, r1 in ((0, capA), (capA, cap)):
                        q = r0
                        while q < r1:
                            m = min(1024, r1 - q)
                            pieces.append((q, m, r0 == 0))
                            q += m

                    NOG = os.environ.get("GNN_NOGATHER")
                    NOE = os.environ.get("GNN_NOELEM")
                    for (q, m, isA) in pieces:
                        b0, b1 = q // 128, (q + m) // 128
                        if NOG:
                            nc.vector.memset(GS[:, b0:b1, :], 0.0)
                            nc.vector.memset(GD[:, b0:b1, :], 0.0)
                        else:
                            if isA:
                                gather_piece(GS, tg_v[:, 0, :], gsix, q, m)
                            else:
                                gather_piece(GS, tg_v[SPLIT:, 0, :], gsix, q, m)
                            gather_piece(GD, tl_v[:, 1, :], gdix, q, m)
                        if not NOE:
                            nc.vector.scalar_tensor_tensor(
                                GS[:, b0:b1, :], GS[:, b0:b1, :], 0.0,
                                GD[:, b0:b1, :], op0=AO.add, op1=AO.add)
                            nc.vector.scalar_tensor_tensor(
                                GS[:, b0:b1, :], GS[:, b0:b1, :], 0.0,
                                BASE[:, b0:b1, :], op0=AO.add, op1=AO.add)
                            nc.scalar.activation(GS[:, b0:b1, :],
                                                 GS[:, b0:b1, :], AF.Relu)

                    pw_tile = psum.tile([128, 512], f32, tag="pw")
                    NOSCAT = os.environ.get("GNN_NOSCAT")
                    for wl, w in enumerate(plan.sw_windows[s]):
                        chunks = window_chunks(s, w)
                        for ci, (b, gb) in enumerate(chunks):
                            if NOSCAT:
                                continue
                            S = pool.tile([128, 128], bf16, tag="S", bufs=4)
                            nc.vector.tensor_scalar(S[:], iota_b[:],
                                                    sohc[:, b:b + 1], None,
                                                    op0=AO.is_equal)
                            nc.tensor.matmul(
                                pw_tile[:, wl * 128:wl * 128 + OUT], S[:],
                                GS[:, b, :OUT], start=(ci == 0),
                                stop=(ci == len(chunks) - 1))
                    for wl, w in enumerate(plan.sw_windows[s]):
                        finalize_window(it, w, pw_tile, wl)
                if it < DEPTH:
                    nc.gpsimd.collective_compute(
                        "AllGather", AO.bypass,
                        replica_groups=[list(range(NCORES))],
                        ins=[tloc[it].opt()], outs=[tglob[it].opt()])

    nc.compile()
    return nc


_CACHE = {}


def kernel(e, p, gamma, beta, W_i, b_i, W_h, b_h, src, dst, num_nodes):
    e = np.asarray(e, np.float32)
    p = np.asarray(p, np.float32)
    src = np.asarray(src, np.int64)
    dst = np.asarray(dst, np.int64)
    N = int(num_nodes)
    OUT = int(np.asarray(W_i).shape[1])

    plan = Plan(src, dst, N)
    sig = plan.signature()
    if sig not in _CACHE:
        _CACHE[sig] = _build(plan, OUT)
    nc = _CACHE[sig]

    per_core = _host_inputs(plan, e, p, src, dst)
    wts = _weight_inputs(plan, np.asarray(gamma), np.asarray(beta),
                         np.asarray(W_i), np.asarray(b_i),
                         np.asarray(W_h), np.asarray(b_h))
    in_maps = [dict(m, **wts) for m in per_core]

    res = run_bass_kernel_spmd(nc, in_maps, core_ids=list(range(NCORES)))
    fn = np.concatenate([np.asarray(res.results[r]["out_fn"],
                                    np.float32)[:plan.NLOC]
                         for r in range(NCORES)], 0)[:N]
    h = np.concatenate([np.asarray(res.results[r]["out_h"],
                                   np.float32)[:plan.NLOC]
                        for r in range(NCORES)], 0)[:N]
    return np.concatenate([fn, h], axis=1)



# revision 10
# speedup vs baseline: 1.3287x; 1.3287x over previous
"""GCN encoder (edge-wise message passing) on 8 Trainium2 NeuronCores.

Strategy (dst-range sharding, v2):
  - Host: sort edges by dst, shard by dst-range (core r owns nodes
    [r*NLOC, (r+1)*NLOC)), group edges into 128-node windows, pad each
    (window, src-half) group to 128-multiples. Degree / index prep on host.
  - Device: BN stats via ACT-accumulate + tiny AllReduce, folded into W_i.
    Pre-pass computes f_e and the loop-invariant per-edge
    base = f_e @ Wh_mid + p*w_p (stored p-major bf16 in HBM), and performs
    the iter-0 scatter from f_e.
    Each iteration: gather g_s[src] (from the AllGathered global src-table)
    and g_d[dst] (from the local dst-table) via dma_gather,
    eh = relu(base + g_s + g_d), scatter-mean via one-hot-moving matmul
    (stationary = eh chunk) accumulating a feature-major node state
    hT [100, 512] in PSUM per superwindow; finalize scales by 1/deg and
    emits the next src/dst tables with a single matmul per window.
    Only the [NLOCP, 128] src-table is AllGathered.
  - Outputs are feature-major [100, NLOCP]; host transposes.
"""
import sys
sys.path.insert(0, "/opt/trn_rl_repo")

import numpy as np
import ml_dtypes
from contextlib import ExitStack

from concourse import bass, bacc, mybir, tile, masks
from concourse.bass_utils import run_bass_kernel_spmd

f32 = mybir.dt.float32
bf16 = mybir.dt.bfloat16
i16 = mybir.dt.int16
i32 = mybir.dt.int32
AO = mybir.AluOpType
AF = mybir.ActivationFunctionType

NCORES = 8
DEPTH = 3
EPS = 1e-5
GW = 4            # windows per superwindow
STAT_SLICE = 1024
GPIECE = 1024    # max slots per dma_gather call (SWDGE ring holds 1024 descs)

bfl = ml_dtypes.bfloat16


def _ru(x, m):
    return (x + m - 1) // m * m


class Plan:
    """Host-side preprocessing: sharding, sorting, padding, index layout."""

    def __init__(self, src, dst, N):
        E = src.shape[0]
        self.N, self.E = N, E
        self.NLOC = (N + NCORES - 1) // NCORES
        self.NWIN = (self.NLOC + 127) // 128
        self.NLOCP = self.NWIN * 128
        self.NGLOB = NCORES * self.NLOCP
        # src-half split: largest rank-multiple of NLOCP that fits int16
        self.SPLIT = min((32768 // self.NLOCP) * self.NLOCP, self.NGLOB)
        assert self.NGLOB - self.SPLIT < 32768

        owner = dst // self.NLOC
        local = dst - owner * self.NLOC
        win = local >> 7
        self.ohval_all = (local & 127).astype(np.float32)
        srcrow = (src // self.NLOC) * self.NLOCP + (src % self.NLOC)
        half = (srcrow >= self.SPLIT).astype(np.int64)
        self.srcrow, self.local, self.owner, self.win, self.half = (
            srcrow, local, owner, win, half)

        key = (owner * self.NWIN + win) * 2 + half
        self.order = np.argsort(key, kind="stable")
        cnt = np.bincount(key, minlength=NCORES * self.NWIN * 2)
        cnt = cnt.reshape(NCORES, self.NWIN, 2)
        self.capA = np.maximum(_ru(cnt[:, :, 0].max(0), 128), 128)
        self.capB = _ru(cnt[:, :, 1].max(0), 128)
        self.cnt = cnt

        # superwindows
        self.NSW = (self.NWIN + GW - 1) // GW
        self.sw_windows = [list(range(s * GW, min((s + 1) * GW, self.NWIN)))
                           for s in range(self.NSW)]
        # slot layout: per sw, [A_w0..A_wk | B_w0..B_wk]
        self.slotA = np.zeros(self.NWIN, np.int64)   # slot offset of A group
        self.slotB = np.zeros(self.NWIN, np.int64)
        self.sw_off = np.zeros(self.NSW + 1, np.int64)
        off = 0
        for s, ws in enumerate(self.sw_windows):
            self.sw_off[s] = off
            a = off
            for w in ws:
                self.slotA[w] = a
                a += self.capA[w]
            for w in ws:
                self.slotB[w] = a
                a += self.capB[w]
            off = a
        self.sw_off[self.NSW] = off
        self.ES = int(off)
        self.sw_capA = [int(sum(self.capA[w] for w in ws))
                        for ws in self.sw_windows]
        self.sw_capB = [int(sum(self.capB[w] for w in ws))
                        for ws in self.sw_windows]
        self.sw_cap = [a + b for a, b in zip(self.sw_capA, self.sw_capB)]
        self.EMAX4 = _ru(max(int((owner == r).sum()) for r in range(NCORES)), 512)
        self.Q4 = self.EMAX4 // 4

    def signature(self):
        return (self.N, self.E, tuple(self.capA), tuple(self.capB))


def _host_inputs(plan, e, p, src, dst):
    """Build the per-core input arrays."""
    NLOC, NWIN, ES = plan.NLOC, plan.NWIN, plan.ES
    order, cnt = plan.order, plan.cnt
    deg = np.maximum(np.bincount(dst, minlength=plan.N), 1).astype(np.float32)
    invd = 1.0 / deg

    in_maps = []
    pos = 0
    # order slices per (r, w, h) in key order
    slices = {}
    for r in range(NCORES):
        for w in range(NWIN):
            for h in range(2):
                c = int(cnt[r, w, h])
                slices[(r, w, h)] = order[pos:pos + c]
                pos += c
    assert pos == plan.E

    for r in range(NCORES):
        efm = np.zeros((34, ES), np.float32)
        efm[32, :] = 1.0
        gsx = np.zeros(ES, np.int16)
        gdx = np.zeros(ES, np.int16)
        ohv = np.full(ES, -5.0, np.float32)
        for w in range(NWIN):
            for h, base_slot in ((0, plan.slotA[w]), (1, plan.slotB[w])):
                idx = slices[(r, w, h)]
                n = idx.shape[0]
                sl = slice(base_slot, base_slot + n)
                efm[0:32, sl] = e[idx].T
                efm[33, sl] = p[idx, 0]
                gsx[sl] = plan.srcrow[idx] - (plan.SPLIT if h else 0)
                gdx[sl] = plan.local[idx]
                ohv[sl] = plan.ohval_all[idx]

        soh = ohv.reshape(-1, 128).T.copy()  # [128, ES//128]
        ivl = np.ones(plan.NLOCP, np.float32)
        lo, hi = r * NLOC, min((r + 1) * NLOC, plan.N)
        ivl[:hi - lo] = invd[lo:hi]
        invdb = np.tile(ivl[None, :], (128, 1))  # [128, NLOCP]

        mask = plan.owner == np.int64(r)
        er = e[mask]
        epad = np.zeros((plan.EMAX4, 32), np.float32)
        epad[:er.shape[0]] = er
        e4 = epad.reshape(4, plan.Q4, 32).transpose(0, 2, 1).reshape(128, plan.Q4)

        # gather idxs: [16, ES//16] wrapped, replicated 8x across partitions
        # (each of the 8 GPSIMD cores reads its own 16-partition copy)
        in_maps.append({
            "efm": efm.astype(bfl),
            "gs_idx": np.tile(gsx.reshape(-1, 16).T, (8, 1)),  # [128, ES//16]
            "gd_idx": np.tile(gdx.reshape(-1, 16).T, (8, 1)),
            "soh": soh,
            "invdb": invdb,
            "e4": e4.astype(bfl),
        })
    return in_maps


def _weight_inputs(plan, gamma, beta, W_i, b_i, W_h, b_h):
    OUT = W_i.shape[1]
    whmid = np.zeros((OUT, 128), np.float32)
    whmid[:, :OUT] = W_h[OUT:2 * OUT]
    wp2 = np.zeros((2, 128), np.float32)
    wp2[1, :OUT] = W_h[2 * OUT]
    whsd = np.zeros((OUT, 256), np.float32)
    whsd[:, 0:OUT] = W_h[0:OUT]
    whsd[:, 128:128 + OUT] = W_h[2 * OUT + 1:3 * OUT + 1]
    bhb = np.zeros((128, 256), np.float32)
    bhb[:, 128:128 + OUT] = b_h[None, :]
    return {
        "W_i": W_i.astype(np.float32),
        "b_i": b_i.reshape(OUT, 1).astype(np.float32),
        "gamma": gamma.reshape(32, 1).astype(np.float32),
        "beta": beta.reshape(32, 1).astype(np.float32),
        "whmid": whmid.astype(bfl),
        "wp2": wp2.astype(bfl),
        "whsd": whsd.astype(bfl),
        "bhb": bhb.astype(bfl),
    }


def _build(plan, OUT):
    """Build + compile the SPMD Bass program for this plan."""
    NWIN, NSW, ES = plan.NWIN, plan.NSW, plan.ES
    NLOCP, NGLOB, SPLIT = plan.NLOCP, plan.NGLOB, plan.SPLIT
    IN = 32

    nc = bacc.Bacc("TRN2", target_bir_lowering=False, debug=False,
                   num_devices=NCORES)

    efm = nc.dram_tensor("efm", [34, ES], bf16, kind="ExternalInput")
    gs_idx = nc.dram_tensor("gs_idx", [128, ES // 16], i16, kind="ExternalInput")
    gd_idx = nc.dram_tensor("gd_idx", [128, ES // 16], i16, kind="ExternalInput")
    soh = nc.dram_tensor("soh", [128, ES // 128], f32, kind="ExternalInput")
    invdb = nc.dram_tensor("invdb", [128, NLOCP], f32, kind="ExternalInput")
    e4 = nc.dram_tensor("e4", [128, plan.Q4], bf16, kind="ExternalInput")
    W_i = nc.dram_tensor("W_i", [IN, OUT], f32, kind="ExternalInput")
    b_i = nc.dram_tensor("b_i", [OUT, 1], f32, kind="ExternalInput")
    gamma = nc.dram_tensor("gamma", [IN, 1], f32, kind="ExternalInput")
    beta = nc.dram_tensor("beta", [IN, 1], f32, kind="ExternalInput")
    whmid = nc.dram_tensor("whmid", [OUT, 128], bf16, kind="ExternalInput")
    wp2 = nc.dram_tensor("wp2", [2, 128], bf16, kind="ExternalInput")
    whsd = nc.dram_tensor("whsd", [OUT, 256], bf16, kind="ExternalInput")
    bhb = nc.dram_tensor("bhb", [128, 256], bf16, kind="ExternalInput")

    out_fnT = nc.dram_tensor("out_fnT", [OUT, NLOCP], f32, kind="ExternalOutput")
    out_hT = nc.dram_tensor("out_hT", [OUT, NLOCP], f32, kind="ExternalOutput")

    inv_E = 1.0 / plan.E

    with tile.TileContext(nc) as tc:
        with ExitStack() as ctx:
            cpool = ctx.enter_context(tc.tile_pool(name="cpool", bufs=1))
            pool = ctx.enter_context(tc.tile_pool(name="pool", bufs=2))
            spool = ctx.enter_context(tc.tile_pool(name="spool", bufs=2))
            psum = ctx.enter_context(tc.tile_pool(name="psum", bufs=2,
                                                  space="PSUM"))
            dram = ctx.enter_context(tc.tile_pool(name="dram", bufs=1,
                                                  space="DRAM"))

            # ---- constants ----
            iota_i = cpool.tile([128, 128], i32)
            nc.gpsimd.iota(iota_i[:], pattern=[[1, 128]], base=0,
                           channel_multiplier=0)
            iota_b = cpool.tile([128, 128], bf16)
            nc.vector.tensor_copy(iota_b[:], iota_i[:])

            whmid_t = cpool.tile([OUT, 128], bf16)
            nc.sync.dma_start(whmid_t[:], whmid[:])
            wp2_t = cpool.tile([34, 128], bf16)
            nc.sync.dma_start(wp2_t[32:34, :], wp2[:])
            whsd_t = cpool.tile([OUT, 256], bf16)
            nc.sync.dma_start(whsd_t[:], whsd[:])
            bhb_t = cpool.tile([128, 256], bf16)
            nc.sync.dma_start(bhb_t[:], bhb[:])
            invdb_t = cpool.tile([128, NLOCP], f32)
            nc.sync.dma_start(invdb_t[:], invdb[:])

            # ---- BN stats: per-core partial sums of e, e^2 ----
            nsl = (plan.Q4 + STAT_SLICE - 1) // STAT_SLICE
            parts = cpool.tile([128, 2 * nsl], f32)
            for s in range(nsl):
                c0, c1 = s * STAT_SLICE, min((s + 1) * STAT_SLICE, plan.Q4)
                esl = spool.tile([128, STAT_SLICE], bf16, tag="esl")
                nc.sync.dma_start(esl[:, :c1 - c0], e4[:, c0:c1])
                junk = spool.tile([128, STAT_SLICE], f32, tag="junk")
                nc.scalar.activation(junk[:, :c1 - c0], esl[:, :c1 - c0],
                                     AF.Copy, accum_out=parts[:, s:s + 1])
                nc.scalar.activation(junk[:, :c1 - c0], esl[:, :c1 - c0],
                                     AF.Square,
                                     accum_out=parts[:, nsl + s:nsl + s + 1])
            sums = cpool.tile([128, 2], f32)
            junk2 = cpool.tile([128, nsl], f32)
            nc.scalar.activation(junk2[:], parts[:, 0:nsl], AF.Copy,
                                 accum_out=sums[:, 0:1])
            nc.scalar.activation(junk2[:], parts[:, nsl:2 * nsl], AF.Copy,
                                 accum_out=sums[:, 1:2])
            ar_in = dram.tile([128, 2], f32)
            ar_out = dram.tile([128, 2], f32)
            nc.sync.dma_start(ar_in[:], sums[:])
            nc.gpsimd.collective_compute(
                "AllReduce", AO.add, replica_groups=[list(range(NCORES))],
                ins=[ar_in.opt()], outs=[ar_out.opt()])
            g4 = cpool.tile([32, 4, 2], f32)
            nc.sync.dma_start(
                g4[:], ar_out[:].rearrange("(g p) k -> p g k", g=4))
            t1 = cpool.tile([32, 2], f32)
            t2 = cpool.tile([32, 2], f32)
            tot = cpool.tile([32, 2], f32)
            nc.vector.tensor_tensor(t1[:], g4[:, 0, :], g4[:, 1, :], AO.add)
            nc.vector.tensor_tensor(t2[:], g4[:, 2, :], g4[:, 3, :], AO.add)
            nc.vector.tensor_tensor(tot[:], t1[:], t2[:], AO.add)
            mu = cpool.tile([32, 1], f32)
            nc.vector.tensor_scalar(mu[:], tot[:, 0:1], inv_E, None, op0=AO.mult)
            ms = cpool.tile([32, 1], f32)
            nc.vector.tensor_scalar(ms[:], tot[:, 1:2], inv_E, None, op0=AO.mult)
            var = cpool.tile([32, 1], f32)
            mu2 = cpool.tile([32, 1], f32)
            nc.vector.tensor_tensor(mu2[:], mu[:], mu[:], AO.mult)
            nc.vector.tensor_tensor(var[:], ms[:], mu2[:], AO.subtract)
            epsb = cpool.tile([32, 1], f32)
            nc.vector.memset(epsb[:], EPS)
            std = cpool.tile([32, 1], f32)
            nc.scalar.activation(std[:], var[:], AF.Sqrt, bias=epsb[:])
            rstd = cpool.tile([32, 1], f32)
            nc.vector.reciprocal(rstd[:], std[:])
            gam_t = cpool.tile([32, 1], f32)
            nc.sync.dma_start(gam_t[:], gamma[:])
            bet_t = cpool.tile([32, 1], f32)
            nc.sync.dma_start(bet_t[:], beta[:])
            a_t = cpool.tile([32, 1], f32)
            nc.vector.tensor_tensor(a_t[:], gam_t[:], rstd[:], AO.mult)
            nma = cpool.tile([32, 1], f32)
            nc.vector.scalar_tensor_tensor(nma[:], mu[:], -1.0, a_t[:],
                                           op0=AO.mult, op1=AO.mult)
            c_t = cpool.tile([32, 1], f32)
            nc.vector.tensor_tensor(c_t[:], bet_t[:], nma[:], AO.add)

            wi_t = cpool.tile([32, OUT], f32)
            nc.sync.dma_start(wi_t[:], W_i[:])
            wif = cpool.tile([32, OUT], f32)
            nc.vector.tensor_scalar(wif[:], wi_t[:], a_t[:], None, op0=AO.mult)
            bi_t = cpool.tile([OUT, 1], f32)
            nc.sync.dma_start(bi_t[:], b_i[:])
            pb = psum.tile([OUT, 1], f32, tag="ptab", bufs=1)
            nc.tensor.matmul(pb[:], wif[:], c_t[:], start=True, stop=True)
            bcol = cpool.tile([OUT, 1], f32)
            nc.vector.tensor_tensor(bcol[:], pb[:], bi_t[:], AO.add)
            scr = dram.tile([OUT, 1], f32)
            nc.sync.dma_start(scr[:], bcol[:])
            # wiaug: [33, 128] (cols 100:128 zero so fee psum is fully written)
            wiaug = cpool.tile([33, 128], bf16)
            nc.vector.memset(wiaug[:], 0.0)
            nc.vector.tensor_copy(wiaug[0:32, :OUT], wif[:])
            nc.gpsimd.dma_start(wiaug[32:33, :OUT],
                                scr[:].rearrange("a b -> b a"))

            # ---- DRAM intermediates ----
            baseH = dram.tile([128, ES], bf16)     # p-major base
            tsrc = [dram.tile([NLOCP, 128], bf16, name=f"tsrc{k}",
                              tag=f"tsrc{k}") for k in range(DEPTH)]
            tdst = [dram.tile([NLOCP, 128], bf16, name=f"tdst{k}",
                              tag=f"tdst{k}") for k in range(DEPTH)]
            tglob = [dram.tile([NGLOB, 128], bf16, name=f"tglob{k}",
                               tag=f"tglob{k}") for k in range(DEPTH)]

            def window_chunks(s, w):
                """(block, ohcol) pairs of window w within superwindow s."""
                o = int(plan.sw_off[s])
                res = []
                for base_slot, cap in ((plan.slotA[w], plan.capA[w]),
                                       (plan.slotB[w], plan.capB[w])):
                    b0 = (int(base_slot) - o) // 128
                    for c in range(int(cap) // 128):
                        res.append(b0 + c)
                return res

            def scatter_sw(it, s, eh_tile, sohc):
                """One-hot scatter of the sw's eh chunks into hT psum.

                eh_tile: [128, nblk*128] bf16 edge-major (stationary slices).
                Returns the [128, 512] psum tile (rows 0:OUT valid).
                """
                pwT = psum.tile([128, 512], f32, tag="pw")
                for wl, w in enumerate(plan.sw_windows[s]):
                    chunks = window_chunks(s, w)
                    for ci, b in enumerate(chunks):
                        S = pool.tile([128, 128], bf16, tag="S", bufs=4)
                        nc.vector.tensor_scalar(S[:], iota_b[:],
                                                sohc[:, b:b + 1], None,
                                                op0=AO.is_equal)
                        nc.tensor.matmul(
                            pwT[0:OUT, wl * 128:(wl + 1) * 128],
                            eh_tile[:, b * 128:b * 128 + OUT],
                            S[:], start=(ci == 0),
                            stop=(ci == len(chunks) - 1))
                return pwT

            def finalize_sw(it, s, pwT):
                """Scale by 1/deg, write outputs / next tables."""
                ws = plan.sw_windows[s]
                wcols = len(ws) * 128
                n0 = ws[0] * 128
                if it == 0 or it == DEPTH:
                    out_t = out_fnT if it == 0 else out_hT
                    hf = pool.tile([OUT, 512], f32, tag="hf")
                    nc.vector.tensor_tensor(hf[:, :wcols], pwT[0:OUT, :wcols],
                                            invdb_t[0:OUT, n0:n0 + wcols],
                                            AO.mult)
                    nc.sync.dma_start(out_t[:, n0:n0 + wcols],
                                      hf[:, :wcols])
                if it == DEPTH:
                    return
                hsc = pool.tile([OUT, 512], bf16, tag="hsc")
                nc.vector.tensor_tensor(hsc[:, :wcols], pwT[0:OUT, :wcols],
                                        invdb_t[0:OUT, n0:n0 + wcols], AO.mult)
                for wl, w in enumerate(ws):
                    ptab = psum.tile([128, 256], f32, tag="ptab", bufs=1)
                    nc.tensor.matmul(ptab[:], hsc[:, wl * 128:(wl + 1) * 128],
                                     whsd_t[:], start=True, stop=True)
                    ttab = pool.tile([128, 256], bf16, tag="ttab", bufs=4)
                    nc.vector.scalar_tensor_tensor(
                        ttab[:], ptab[:], 0.0, bhb_t[:],
                        op0=AO.add, op1=AO.add)
                    nc.sync.dma_start(tsrc[it][w * 128:(w + 1) * 128, :],
                                      ttab[:, 0:128])
                    nc.sync.dma_start(tdst[it][w * 128:(w + 1) * 128, :],
                                      ttab[:, 128:256])

            # ---- pre-pass + iter 0 ----
            for s in range(NSW):
                cap = plan.sw_cap[s]
                nblk = cap // 128
                o0 = int(plan.sw_off[s])
                efm_t = pool.tile([34, cap], bf16, tag="big0")
                nc.sync.dma_start(efm_t[:], efm[:, o0:o0 + cap])
                sohc = pool.tile([128, nblk], f32, tag="sohc")
                nc.sync.dma_start(sohc[:], soh[:, o0 // 128:o0 // 128 + nblk])
                feT = pool.tile([OUT, cap], bf16, tag="big1")
                baseC = pool.tile([128, cap], bf16, tag="big2")
                fee = pool.tile([128, cap], bf16, tag="fee")

                for g0 in range(0, cap, 512):
                    g1 = min(g0 + 512, cap)
                    p1 = psum.tile([OUT, 512], f32, tag="p1", bufs=1)
                    nc.tensor.matmul(p1[:, :g1 - g0], wiaug[:, :OUT],
                                     efm_t[0:33, g0:g1], start=True, stop=True)
                    nc.scalar.activation(feT[:, g0:g1], p1[:, :g1 - g0], AF.Relu)

                # base + fee per 4-chunk group
                for g0 in range(0, cap, 512):
                    g1 = min(g0 + 512, cap)
                    pbs = psum.tile([128, 512], f32, tag="pbase")
                    pfe = psum.tile([128, 512], f32, tag="pfee")
                    for c0 in range(g0, g1, 128):
                        sl = slice(c0, c0 + 128)
                        cc = c0 - g0
                        nc.tensor.matmul(pbs[:, cc:cc + 128], feT[:, sl],
                                         whmid_t[:], start=True, stop=False)
                        nc.tensor.matmul(pbs[:, cc:cc + 128],
                                         efm_t[32:34, sl], wp2_t[32:34, :],
                                         start=False, stop=True)
                        nc.tensor.matmul(pfe[:, cc:cc + 128],
                                         efm_t[0:33, sl], wiaug[:],
                                         start=True, stop=True)
                    nc.vector.tensor_copy(baseC[:, g0:g1], pbs[:, :g1 - g0])
                    nc.scalar.activation(fee[:, g0:g1], pfe[:, :g1 - g0],
                                         AF.Relu)
                nc.sync.dma_start(baseH[:, o0:o0 + cap], baseC[:])

                pwT = scatter_sw(0, s, fee, sohc)
                finalize_sw(0, s, pwT)

            nc.gpsimd.collective_compute(
                "AllGather", AO.bypass,
                replica_groups=[list(range(NCORES))],
                ins=[tsrc[0].opt()], outs=[tglob[0].opt()])

            # ---- iterations 1..DEPTH ----
            for it in range(1, DEPTH + 1):
                for s in range(NSW):
                    cap = plan.sw_cap[s]
                    nblk = cap // 128
                    capA = plan.sw_capA[s]
                    o0 = int(plan.sw_off[s])
                    GD = pool.tile([128, nblk, 128], bf16, tag="big0")
                    GS = pool.tile([128, nblk, 128], bf16, tag="big1")
                    BASE = pool.tile([128, cap], bf16, tag="big2")
                    nc.sync.dma_start(BASE[:], baseH[:, o0:o0 + cap])
                    sohc = pool.tile([128, nblk], f32, tag="sohc")
                    nc.sync.dma_start(sohc[:],
                                      soh[:, o0 // 128:o0 // 128 + nblk])
                    gdix = pool.tile([128, cap // 16], i16, tag="gdix")
                    nc.sync.dma_start(gdix[:],
                                      gd_idx[:, o0 // 16:(o0 + cap) // 16])
                    gsix = pool.tile([128, cap // 16], i16, tag="gsix")
                    nc.sync.dma_start(gsix[:],
                                      gs_idx[:, o0 // 16:(o0 + cap) // 16])

                    # pieces of <=GPIECE slots, within A then B regions
                    pieces = []
                    for r0, r1 in ((0, capA), (capA, cap)):
                        q = r0
                        while q < r1:
                            m = min(GPIECE, r1 - q)
                            pieces.append((q, m, r0 == 0))
                            q += m

                    def gather_piece(dst_tile, src_view, ix_tile, off, m):
                        b0 = off // 128
                        nc.gpsimd.dma_gather(
                            dst_tile[:, b0:b0 + m // 128, :], src_view,
                            ix_tile[:16, off // 16:(off + m) // 16],
                            m, m, 128)

                    # GD gathers first: they don't depend on the AllGather,
                    # so they can run underneath it.
                    for (q, m, isA) in pieces:
                        gather_piece(GD, tdst[it - 1][:], gdix, q, m)
                    for (q, m, isA) in pieces:
                        if isA:
                            gather_piece(GS, tglob[it - 1][:], gsix, q, m)
                        else:
                            gather_piece(GS, tglob[it - 1][SPLIT:, :],
                                         gsix, q, m)

                    for (q, m, isA) in pieces:
                        b0, b1 = q // 128, (q + m) // 128
                        nc.vector.scalar_tensor_tensor(
                            GS[:, b0:b1, :], GS[:, b0:b1, :], 0.0,
                            GD[:, b0:b1, :], op0=AO.add, op1=AO.add)
                        nc.vector.scalar_tensor_tensor(
                            GS[:, b0:b1, :], GS[:, b0:b1, :], 0.0,
                            BASE[:, q:q + m].rearrange(
                                "p (b c) -> p b c", c=128),
                            op0=AO.add, op1=AO.add)
                        nc.scalar.activation(GS[:, b0:b1, :],
                                             GS[:, b0:b1, :], AF.Relu)

                    ehv = GS[:].rearrange("p b c -> p (b c)")
                    pwT = scatter_sw(it, s, ehv, sohc)
                    finalize_sw(it, s, pwT)
                if it < DEPTH:
                    nc.gpsimd.collective_compute(
                        "AllGather", AO.bypass,
                        replica_groups=[list(range(NCORES))],
                        ins=[tsrc[it].opt()], outs=[tglob[it].opt()])

    nc.compile()
    return nc


_CACHE = {}


def kernel(e, p, gamma, beta, W_i, b_i, W_h, b_h, src, dst, num_nodes):
    e = np.asarray(e, np.float32)
    p = np.asarray(p, np.float32)
    src = np.asarray(src, np.int64)
    dst = np.asarray(dst, np.int64)
    N = int(num_nodes)
    OUT = int(np.asarray(W_i).shape[1])

    plan = Plan(src, dst, N)
    sig = plan.signature()
    if sig not in _CACHE:
        _CACHE[sig] = _build(plan, OUT)
    nc = _CACHE[sig]

    per_core = _host_inputs(plan, e, p, src, dst)
    wts = _weight_inputs(plan, np.asarray(gamma), np.asarray(beta),
                         np.asarray(W_i), np.asarray(b_i),
                         np.asarray(W_h), np.asarray(b_h))
    in_maps = [dict(m, **wts) for m in per_core]

    res = run_bass_kernel_spmd(nc, in_maps, core_ids=list(range(NCORES)))
    fn = np.concatenate([np.asarray(res.results[r]["out_fnT"],
                                    np.float32)[:, :plan.NLOC].T
                         for r in range(NCORES)], 0)[:N]
    h = np.concatenate([np.asarray(res.results[r]["out_hT"],
                                   np.float32)[:, :plan.NLOC].T
                        for r in range(NCORES)], 0)[:N]
    return np.concatenate([fn, h], axis=1)


# revision 22
# speedup vs baseline: 1.5351x; 1.1554x over previous
"""GCN encoder (edge-wise message passing) on 8 Trainium2 NeuronCores.

Strategy (dst-range sharding, v2):
  - Host: sort edges by dst, shard by dst-range (core r owns nodes
    [r*NLOC, (r+1)*NLOC)), group edges into 128-node windows, pad each
    (window, src-half) group to 128-multiples. Degree / index prep on host.
  - Device: BN stats via ACT-accumulate + tiny AllReduce, folded into W_i.
    Pre-pass computes f_e and the loop-invariant per-edge
    base = f_e @ Wh_mid + p*w_p (stored p-major bf16 in HBM), and performs
    the iter-0 scatter from f_e.
    Each iteration: gather g_s[src] (from the AllGathered global src-table)
    and g_d[dst] (from the local dst-table) via dma_gather,
    eh = relu(base + g_s + g_d), scatter-mean via one-hot-moving matmul
    (stationary = eh chunk) accumulating a feature-major node state
    hT [100, 512] in PSUM per superwindow; finalize scales by 1/deg and
    emits the next src/dst tables with a single matmul per window.
    Only the [NLOCP, 128] src-table is AllGathered.
  - Outputs are feature-major [100, NLOCP]; host transposes.
"""
import sys
sys.path.insert(0, "/opt/trn_rl_repo")

import numpy as np
import ml_dtypes
from contextlib import ExitStack

from concourse import bass, bacc, mybir, tile, masks
from concourse.bass_utils import run_bass_kernel_spmd

f32 = mybir.dt.float32
bf16 = mybir.dt.bfloat16
i16 = mybir.dt.int16
i32 = mybir.dt.int32
AO = mybir.AluOpType
AF = mybir.ActivationFunctionType

NCORES = 8
DEPTH = 3
EPS = 1e-5
GW = 4            # windows per superwindow
STAT_SLICE = 1024
GPIECE = 1024    # max slots per dma_gather call (SWDGE ring holds 1024 descs)

bfl = ml_dtypes.bfloat16


def _ru(x, m):
    return (x + m - 1) // m * m


class Plan:
    """Host-side preprocessing: sharding, sorting, padding, index layout."""

    def __init__(self, src, dst, N):
        E = src.shape[0]
        self.N, self.E = N, E
        self.NLOC = (N + NCORES - 1) // NCORES
        self.NWIN = (self.NLOC + 127) // 128
        self.NLOCP = self.NWIN * 128
        self.NGLOB = NCORES * self.NLOCP
        # src-half split: largest rank-multiple of NLOCP that fits int16
        self.SPLIT = min((32768 // self.NLOCP) * self.NLOCP, self.NGLOB)
        assert self.NGLOB - self.SPLIT < 32768

        owner = dst // self.NLOC
        local = dst - owner * self.NLOC
        win = local >> 7
        self.ohval_all = (local & 127).astype(np.float32)
        srcrow = (src // self.NLOC) * self.NLOCP + (src % self.NLOC)
        half = (srcrow >= self.SPLIT).astype(np.int64)
        self.srcrow, self.local, self.owner, self.win, self.half = (
            srcrow, local, owner, win, half)

        key = (owner * self.NWIN + win) * 2 + half
        self.order = np.argsort(key, kind="stable")
        cnt = np.bincount(key, minlength=NCORES * self.NWIN * 2)
        cnt = cnt.reshape(NCORES, self.NWIN, 2)
        self.capA = np.maximum(_ru(cnt[:, :, 0].max(0), 128), 128)
        self.capB = _ru(cnt[:, :, 1].max(0), 128)
        self.cnt = cnt

        # superwindows
        self.NSW = (self.NWIN + GW - 1) // GW
        self.sw_windows = [list(range(s * GW, min((s + 1) * GW, self.NWIN)))
                           for s in range(self.NSW)]
        # slot layout: per sw, [A_w0..A_wk | B_w0..B_wk]
        self.slotA = np.zeros(self.NWIN, np.int64)   # slot offset of A group
        self.slotB = np.zeros(self.NWIN, np.int64)
        self.sw_off = np.zeros(self.NSW + 1, np.int64)
        off = 0
        for s, ws in enumerate(self.sw_windows):
            self.sw_off[s] = off
            a = off
            for w in ws:
                self.slotA[w] = a
                a += self.capA[w]
                self.slotB[w] = a
                a += self.capB[w]
            off = a
        self.sw_off[self.NSW] = off
        self.ES = int(off)
        self.sw_capA = [int(sum(self.capA[w] for w in ws))
                        for ws in self.sw_windows]
        self.sw_capB = [int(sum(self.capB[w] for w in ws))
                        for ws in self.sw_windows]
        self.sw_cap = [a + b for a, b in zip(self.sw_capA, self.sw_capB)]
        self.EMAX4 = _ru(max(int((owner == r).sum()) for r in range(NCORES)), 512)
        self.Q4 = self.EMAX4 // 4

    def signature(self):
        return (self.N, self.E, tuple(self.capA), tuple(self.capB))


def _host_inputs(plan, e, p, src, dst):
    """Build the per-core input arrays."""
    NLOC, NWIN, ES = plan.NLOC, plan.NWIN, plan.ES
    order, cnt = plan.order, plan.cnt
    deg = np.maximum(np.bincount(dst, minlength=plan.N), 1).astype(np.float32)
    invd = 1.0 / deg

    in_maps = []
    pos = 0
    # order slices per (r, w, h) in key order
    slices = {}
    for r in range(NCORES):
        for w in range(NWIN):
            for h in range(2):
                c = int(cnt[r, w, h])
                slices[(r, w, h)] = order[pos:pos + c]
                pos += c
    assert pos == plan.E

    for r in range(NCORES):
        efm = np.zeros((34, ES), np.float32)
        efm[32, :] = 1.0
        gsx = np.zeros(ES, np.int16)
        gdx = np.zeros(ES, np.int16)
        ohv = np.full(ES, -5.0, np.float32)
        for w in range(NWIN):
            for h, base_slot in ((0, plan.slotA[w]), (1, plan.slotB[w])):
                idx = slices[(r, w, h)]
                n = idx.shape[0]
                sl = slice(base_slot, base_slot + n)
                efm[0:32, sl] = e[idx].T
                efm[33, sl] = p[idx, 0]
                gsx[sl] = plan.srcrow[idx] - (plan.SPLIT if h else 0)
                gdx[sl] = plan.local[idx]
                ohv[sl] = plan.ohval_all[idx]

        soh = ohv.reshape(-1, 128).T.copy()  # [128, ES//128]
        ivl = np.ones(plan.NLOCP, np.float32)
        lo, hi = r * NLOC, min((r + 1) * NLOC, plan.N)
        ivl[:hi - lo] = invd[lo:hi]
        invdb = np.tile(ivl[None, :], (128, 1))  # [128, NLOCP]

        mask = plan.owner == np.int64(r)
        er = e[mask]
        epad = np.zeros((plan.EMAX4, 32), np.float32)
        epad[:er.shape[0]] = er
        e4 = epad.reshape(4, plan.Q4, 32).transpose(0, 2, 1).reshape(128, plan.Q4)

        # gather idxs: [16, ES//16] wrapped, replicated 8x across partitions
        # (each of the 8 GPSIMD cores reads its own 16-partition copy)
        in_maps.append({
            "efm": efm.astype(bfl),
            "gs_idx": np.tile(gsx.reshape(-1, 16).T, (8, 1)),  # [128, ES//16]
            "soh": soh,
            "stf": (ohv[None, :] == np.arange(128, dtype=np.float32)[:, None]
                    ).astype(bfl),
            "invdb": invdb,
            "e4": e4.astype(bfl),
        })
    return in_maps


def _weight_inputs(plan, gamma, beta, W_i, b_i, W_h, b_h):
    OUT = W_i.shape[1]
    whmid = np.zeros((OUT + 1, 128), np.float32)
    whmid[:OUT, :OUT] = W_h[OUT:2 * OUT]
    whmid[OUT, :OUT] = W_h[2 * OUT]
    whsd = np.zeros((OUT, 256), np.float32)
    whsd[:, 0:OUT] = W_h[0:OUT]
    whsd[:, 128:128 + OUT] = W_h[2 * OUT + 1:3 * OUT + 1]
    bhb = np.zeros((128, 128), np.float32)
    bhb[:, 0:OUT] = b_h[None, :]
    return {
        "W_i": W_i.astype(np.float32),
        "b_i": b_i.reshape(OUT, 1).astype(np.float32),
        "gamma": gamma.reshape(32, 1).astype(np.float32),
        "beta": beta.reshape(32, 1).astype(np.float32),
        "whmid": whmid.astype(bfl),
        "whsd": whsd.astype(bfl),
        "bhb": bhb.astype(bfl),
    }


def _build(plan, OUT):
    """Build + compile the SPMD Bass program for this plan."""
    NWIN, NSW, ES = plan.NWIN, plan.NSW, plan.ES
    NLOCP, NGLOB, SPLIT = plan.NLOCP, plan.NGLOB, plan.SPLIT
    IN = 32

    nc = bacc.Bacc("TRN2", target_bir_lowering=False, debug=False,
                   num_devices=NCORES)

    efm = nc.dram_tensor("efm", [34, ES], bf16, kind="ExternalInput")
    gs_idx = nc.dram_tensor("gs_idx", [128, ES // 16], i16, kind="ExternalInput")
    soh = nc.dram_tensor("soh", [128, ES // 128], f32, kind="ExternalInput")
    stf = nc.dram_tensor("stf", [128, ES], bf16, kind="ExternalInput")
    invdb = nc.dram_tensor("invdb", [128, NLOCP], f32, kind="ExternalInput")
    e4 = nc.dram_tensor("e4", [128, plan.Q4], bf16, kind="ExternalInput")
    W_i = nc.dram_tensor("W_i", [IN, OUT], f32, kind="ExternalInput")
    b_i = nc.dram_tensor("b_i", [OUT, 1], f32, kind="ExternalInput")
    gamma = nc.dram_tensor("gamma", [IN, 1], f32, kind="ExternalInput")
    beta = nc.dram_tensor("beta", [IN, 1], f32, kind="ExternalInput")
    whmid = nc.dram_tensor("whmid", [OUT + 1, 128], bf16,
                           kind="ExternalInput")
    whsd = nc.dram_tensor("whsd", [OUT, 256], bf16, kind="ExternalInput")
    bhb = nc.dram_tensor("bhb", [128, 128], bf16, kind="ExternalInput")

    out_fnT = nc.dram_tensor("out_fnT", [OUT, NLOCP], f32, kind="ExternalOutput")
    out_hT = nc.dram_tensor("out_hT", [OUT, NLOCP], f32, kind="ExternalOutput")

    inv_E = 1.0 / plan.E

    with tile.TileContext(nc) as tc:
        with ExitStack() as ctx:
            cpool = ctx.enter_context(tc.tile_pool(name="cpool", bufs=1))
            pool = ctx.enter_context(tc.tile_pool(name="pool", bufs=2))
            spool = ctx.enter_context(tc.tile_pool(name="spool", bufs=2))
            psum = ctx.enter_context(tc.tile_pool(name="psum", bufs=2,
                                                  space="PSUM"))
            dram = ctx.enter_context(tc.tile_pool(name="dram", bufs=1,
                                                  space="DRAM"))

            # ---- constants ----
            iota_i = cpool.tile([128, 128], i32)
            nc.gpsimd.iota(iota_i[:], pattern=[[1, 128]], base=0,
                           channel_multiplier=0)
            iota_b = cpool.tile([128, 128], bf16)
            nc.vector.tensor_copy(iota_b[:], iota_i[:])

            identf = cpool.tile([128, 128], f32)
            masks.make_identity(nc, identf[:])
            identb = cpool.tile([128, 128], bf16)
            nc.vector.tensor_copy(identb[:], identf[:])

            whmid_t = cpool.tile([OUT + 1, 128], bf16)
            nc.sync.dma_start(whmid_t[:], whmid[:])
            whsd_t = cpool.tile([OUT, 256], bf16)
            nc.sync.dma_start(whsd_t[:], whsd[:])
            bhb_t = cpool.tile([128, 128], bf16)
            nc.sync.dma_start(bhb_t[:], bhb[:])
            tblD_sb = cpool.tile([128, NWIN * 128], bf16)

            # ---- BN stats: per-core partial sums of e, e^2 ----
            nsl = (plan.Q4 + STAT_SLICE - 1) // STAT_SLICE
            parts = cpool.tile([128, 2 * nsl], f32)
            for s in range(nsl):
                c0, c1 = s * STAT_SLICE, min((s + 1) * STAT_SLICE, plan.Q4)
                esl = spool.tile([128, STAT_SLICE], bf16, tag="esl")
                nc.sync.dma_start(esl[:, :c1 - c0], e4[:, c0:c1])
                junk = spool.tile([128, STAT_SLICE], f32, tag="junk")
                nc.scalar.activation(junk[:, :c1 - c0], esl[:, :c1 - c0],
                                     AF.Copy, accum_out=parts[:, s:s + 1])
                nc.scalar.activation(junk[:, :c1 - c0], esl[:, :c1 - c0],
                                     AF.Square,
                                     accum_out=parts[:, nsl + s:nsl + s + 1])
            sums = cpool.tile([128, 2], f32)
            junk2 = cpool.tile([128, nsl], f32)
            nc.scalar.activation(junk2[:], parts[:, 0:nsl], AF.Copy,
                                 accum_out=sums[:, 0:1])
            nc.scalar.activation(junk2[:], parts[:, nsl:2 * nsl], AF.Copy,
                                 accum_out=sums[:, 1:2])
            ar_in = dram.tile([128, 2], f32)
            ar_out = dram.tile([128, 2], f32)
            nc.sync.dma_start(ar_in[:], sums[:])
            nc.gpsimd.collective_compute(
                "AllReduce", AO.add, replica_groups=[list(range(NCORES))],
                ins=[ar_in.opt()], outs=[ar_out.opt()])
            g4 = cpool.tile([32, 4, 2], f32)
            nc.sync.dma_start(
                g4[:], ar_out[:].rearrange("(g p) k -> p g k", g=4))
            t1 = cpool.tile([32, 2], f32)
            t2 = cpool.tile([32, 2], f32)
            tot = cpool.tile([32, 2], f32)
            nc.vector.tensor_tensor(t1[:], g4[:, 0, :], g4[:, 1, :], AO.add)
            nc.vector.tensor_tensor(t2[:], g4[:, 2, :], g4[:, 3, :], AO.add)
            nc.vector.tensor_tensor(tot[:], t1[:], t2[:], AO.add)
            mu = cpool.tile([32, 1], f32)
            nc.vector.tensor_scalar(mu[:], tot[:, 0:1], inv_E, None, op0=AO.mult)
            ms = cpool.tile([32, 1], f32)
            nc.vector.tensor_scalar(ms[:], tot[:, 1:2], inv_E, None, op0=AO.mult)
            var = cpool.tile([32, 1], f32)
            mu2 = cpool.tile([32, 1], f32)
            nc.vector.tensor_tensor(mu2[:], mu[:], mu[:], AO.mult)
            nc.vector.tensor_tensor(var[:], ms[:], mu2[:], AO.subtract)
            epsb = cpool.tile([32, 1], f32)
            nc.vector.memset(epsb[:], EPS)
            std = cpool.tile([32, 1], f32)
            nc.scalar.activation(std[:], var[:], AF.Sqrt, bias=epsb[:])
            rstd = cpool.tile([32, 1], f32)
            nc.vector.reciprocal(rstd[:], std[:])
            gam_t = cpool.tile([32, 1], f32)
            nc.sync.dma_start(gam_t[:], gamma[:])
            bet_t = cpool.tile([32, 1], f32)
            nc.sync.dma_start(bet_t[:], beta[:])
            a_t = cpool.tile([32, 1], f32)
            nc.vector.tensor_tensor(a_t[:], gam_t[:], rstd[:], AO.mult)
            nma = cpool.tile([32, 1], f32)
            nc.vector.scalar_tensor_tensor(nma[:], mu[:], -1.0, a_t[:],
                                           op0=AO.mult, op1=AO.mult)
            c_t = cpool.tile([32, 1], f32)
            nc.vector.tensor_tensor(c_t[:], bet_t[:], nma[:], AO.add)

            wi_t = cpool.tile([32, OUT], f32)
            nc.sync.dma_start(wi_t[:], W_i[:])
            wif = cpool.tile([32, OUT], f32)
            nc.vector.tensor_scalar(wif[:], wi_t[:], a_t[:], None, op0=AO.mult)
            bi_t = cpool.tile([OUT, 1], f32)
            nc.sync.dma_start(bi_t[:], b_i[:])
            pb = psum.tile([OUT, 1], f32, tag="ptab", bufs=1)
            nc.tensor.matmul(pb[:], wif[:], c_t[:], start=True, stop=True)
            bcol = cpool.tile([OUT, 1], f32)
            nc.vector.tensor_tensor(bcol[:], pb[:], bi_t[:], AO.add)
            scr = dram.tile([OUT, 1], f32)
            nc.sync.dma_start(scr[:], bcol[:])
            # wiaug: [33, 128] (cols 100:128 zero so fee psum is fully written)
            wiaug = cpool.tile([33, 128], bf16)
            nc.vector.memset(wiaug[:], 0.0)
            nc.vector.tensor_copy(wiaug[0:32, :OUT], wif[:])
            nc.gpsimd.dma_start(wiaug[32:33, :OUT],
                                scr[:].rearrange("a b -> b a"))

            # ---- DRAM intermediates ----
            baseH = dram.tile([128, (ES // 128) * OUT], bf16)
            tsrc = [dram.tile([NLOCP, 128], bf16, name=f"tsrc{k}",
                              tag=f"tsrc{k}") for k in range(DEPTH)]
            tglob = [dram.tile([NGLOB, 128], bf16, name=f"tglob{k}",
                               tag=f"tglob{k}") for k in range(DEPTH)]

            def sw_blocks(s):
                """Per sw-local block: (w, wl, start_flag, stop_flag)."""
                o0 = int(plan.sw_off[s])
                nblk = plan.sw_cap[s] // 128
                info = [None] * nblk
                for wl, w in enumerate(plan.sw_windows[s]):
                    blocks = []
                    for base_slot, capw in ((plan.slotA[w], plan.capA[w]),
                                            (plan.slotB[w], plan.capB[w])):
                        b0 = (int(base_slot) - o0) // 128
                        blocks += list(range(b0, b0 + int(capw) // 128))
                    for i, b in enumerate(blocks):
                        info[b] = (w, wl, i == 0, i == len(blocks) - 1)
                return info

            def build_Sw(sohc, b0, nb):
                """Edge-major one-hots for blocks [b0, b0+nb), one per block
                (DVE stride-0 broadcast APs are not supported by hardware)."""
                Sw = pool.tile([128, 4, 128], bf16, tag="S", bufs=4)
                for i in range(nb):
                    nc.vector.tensor_scalar(Sw[:, i, :], iota_b[:],
                                            sohc[:, b0 + i:b0 + i + 1], None,
                                            op0=AO.is_equal)
                return Sw

            def finalize_sw(it, s, pwT, ivd):
                """Scale by 1/deg, write outputs / next tables."""
                ws = plan.sw_windows[s]
                wcols = len(ws) * 128
                n0 = ws[0] * 128
                if it == 0 or it == DEPTH:
                    out_t = out_fnT if it == 0 else out_hT
                    hf = pool.tile([OUT, 512], f32, tag="hf")
                    nc.vector.tensor_tensor(hf[:, :wcols], pwT[0:OUT, :wcols],
                                            ivd[0:OUT, :wcols], AO.mult)
                    nc.sync.dma_start(out_t[:, n0:n0 + wcols],
                                      hf[:, :wcols])
                if it == DEPTH:
                    return
                hsc = pool.tile([OUT, 512], bf16, tag="hsc")
                nc.vector.tensor_tensor(hsc[:, :wcols], pwT[0:OUT, :wcols],
                                        ivd[0:OUT, :wcols], AO.mult)
                for wl, w in enumerate(ws):
                    ptab = psum.tile([128, 256], f32, tag="ptab", bufs=1)
                    nc.tensor.matmul(ptab[:], hsc[:, wl * 128:(wl + 1) * 128],
                                     whsd_t[:], start=True, stop=True)
                    ttab = pool.tile([128, 128], bf16, tag="ttab", bufs=4)
                    nc.scalar.copy(ttab[:], ptab[:, 0:128])
                    nc.vector.scalar_tensor_tensor(
                        tblD_sb[:, w * 128:(w + 1) * 128], ptab[:, 128:256],
                        0.0, bhb_t[:], op0=AO.add, op1=AO.add)
                    nc.sync.dma_start(tsrc[it][w * 128:(w + 1) * 128, :],
                                      ttab[:])

            # ---- pre-pass + iter 0 ----
            for s in range(NSW):
                cap = plan.sw_cap[s]
                nblk = cap // 128
                o0 = int(plan.sw_off[s])
                n0 = plan.sw_windows[s][0] * 128
                wcols = len(plan.sw_windows[s]) * 128
                binfo = sw_blocks(s)
                efm_t = pool.tile([34, cap], bf16, tag="big0")
                nc.sync.dma_start(efm_t[:], efm[:, o0:o0 + cap])
                sohc = pool.tile([128, nblk], f32, tag="sohc")
                nc.sync.dma_start(sohc[:], soh[:, o0 // 128:o0 // 128 + nblk])
                ivd = pool.tile([128, 512], f32, tag="ivd")
                nc.sync.dma_start(ivd[:, :wcols], invdb[:, n0:n0 + wcols])
                feT = pool.tile([OUT + 1, cap], bf16, tag="big1")
                baseC = pool.tile([128, nblk, OUT], bf16, tag="big2")

                for g0 in range(0, cap, 512):
                    g1 = min(g0 + 512, cap)
                    p1 = psum.tile([OUT, 512], f32, tag="p1", bufs=1)
                    nc.tensor.matmul(p1[:, :g1 - g0], wiaug[:, :OUT],
                                     efm_t[0:33, g0:g1], start=True, stop=True)
                    nc.scalar.activation(feT[0:OUT, g0:g1], p1[:, :g1 - g0],
                                         AF.Relu)

                nc.sync.dma_start(feT[OUT:OUT + 1, :], efm_t[33:34, :])
                pwT = psum.tile([128, 512], f32, tag="pw")
                for g0 in range(0, cap, 512):
                    g1 = min(g0 + 512, cap)
                    b0, b1 = g0 // 128, g1 // 128
                    pbs = psum.tile([128, 4, 128], f32, tag="pbase")
                    pfe = psum.tile([128, 512], f32, tag="pfee")
                    for c0 in range(g0, g1, 128):
                        sl = slice(c0, c0 + 128)
                        ci = (c0 - g0) // 128
                        cc = c0 - g0
                        nc.tensor.matmul(pbs[:, ci, :], feT[:, sl],
                                         whmid_t[:], start=True, stop=True)
                        nc.tensor.matmul(pfe[:, cc:cc + 128],
                                         efm_t[0:33, sl], wiaug[:],
                                         start=True, stop=True)
                    nc.vector.tensor_copy(baseC[:, b0:b1, :],
                                          pbs[:, 0:b1 - b0, 0:OUT])
                    fee = pool.tile([128, 512], bf16, tag="feeg")
                    nc.scalar.activation(fee[:, :g1 - g0], pfe[:, :g1 - g0],
                                         AF.Relu)
                    Sw = build_Sw(sohc, b0, b1 - b0)
                    for b in range(b0, b1):
                        w, wl, st, sp = binfo[b]
                        cc = b * 128 - g0
                        nc.tensor.matmul(pwT[0:OUT, wl * 128:(wl + 1) * 128],
                                         fee[:, cc:cc + OUT], Sw[:, b - b0, :],
                                         start=st, stop=sp)
                ob = (o0 // 128) * OUT
                nc.sync.dma_start(baseH[:, ob:ob + nblk * OUT], baseC[:])
                finalize_sw(0, s, pwT, ivd)

            nc.gpsimd.collective_compute(
                "AllGather", AO.bypass,
                replica_groups=[list(range(NCORES))],
                ins=[tsrc[0].opt()], outs=[tglob[0].opt()])

            # ---- iterations 1..DEPTH ----
            for it in range(1, DEPTH + 1):
                for s in range(NSW):
                    cap = plan.sw_cap[s]
                    nblk = cap // 128
                    capA = plan.sw_capA[s]
                    o0 = int(plan.sw_off[s])
                    n0 = plan.sw_windows[s][0] * 128
                    wcols = len(plan.sw_windows[s]) * 128
                    binfo = sw_blocks(s)
                    GS = pool.tile([128, nblk, 128], bf16, tag="big1")
                    BASE = pool.tile([128, nblk, OUT], bf16, tag="big2")
                    ob = (o0 // 128) * OUT
                    nc.sync.dma_start(BASE[:], baseH[:, ob:ob + nblk * OUT])
                    sohc = pool.tile([128, nblk], f32, tag="sohc")
                    nc.sync.dma_start(sohc[:],
                                      soh[:, o0 // 128:o0 // 128 + nblk])
                    ivd = pool.tile([128, 512], f32, tag="ivd")
                    nc.sync.dma_start(ivd[:, :wcols], invdb[:, n0:n0 + wcols])
                    STt = pool.tile([128, cap], bf16, tag="STt")
                    nc.sync.dma_start(STt[:], stf[:, o0:o0 + cap])
                    gsix = pool.tile([128, cap // 16], i16, tag="gsix")
                    nc.sync.dma_start(gsix[:],
                                      gs_idx[:, o0 // 16:(o0 + cap) // 16])

                    # pieces of <=GPIECE slots within each (window, half)
                    # region run (layout is window-major: A_w B_w A_w+1 ...)
                    pieces = []
                    for w in plan.sw_windows[s]:
                        for base_slot, capw, isA in (
                                (plan.slotA[w], plan.capA[w], True),
                                (plan.slotB[w], plan.capB[w], False)):
                            q = int(base_slot) - o0
                            r1 = q + int(capw)
                            while q < r1:
                                m = min(GPIECE, r1 - q)
                                pieces.append((q, m, isA))
                                q += m

                    for (q, m, isA) in pieces:
                        b0 = q // 128
                        src_v = (tglob[it - 1][:] if isA
                                 else tglob[it - 1][SPLIT:, :])
                        nc.gpsimd.dma_gather(
                            GS[:, b0:b0 + m // 128, :], src_v,
                            gsix[:, q // 16:(q + m) // 16], m, m, 128)

                    GSf = GS[:].rearrange("p b c -> p (b c)")
                    pwT = psum.tile([128, 512], f32, tag="pw")
                    for g0 in range(0, cap, 512):
                        g1 = min(g0 + 512, cap)
                        gcols = g1 - g0
                        b0, b1 = g0 // 128, g1 // 128
                        nc.vector.scalar_tensor_tensor(
                            GS[:, b0:b1, 0:OUT], GS[:, b0:b1, 0:OUT], 0.0,
                            BASE[:, b0:b1, :], op0=AO.add, op1=AO.add)
                        u = psum.tile([128, 512], f32, tag="pbase")
                        nc.tensor.matmul(u[:, :gcols], identb[:],
                                         GSf[:, g0:g1], start=True, stop=True)
                        for b in range(b0, b1):
                            w = binfo[b][0]
                            cc = b * 128 - g0
                            nc.tensor.matmul(
                                u[:, cc:cc + 128], STt[:, b * 128:b * 128 + 128],
                                tblD_sb[:, w * 128:(w + 1) * 128],
                                start=False, stop=True, skip_group_check=True)
                        nc.scalar.activation(GSf[:, g0:g1], u[:, :gcols],
                                             AF.Relu)
                        Sw = build_Sw(sohc, b0, b1 - b0)
                        for b in range(b0, b1):
                            w, wl, st, sp = binfo[b]
                            nc.tensor.matmul(
                                pwT[0:OUT, wl * 128:(wl + 1) * 128],
                                GSf[:, b * 128:b * 128 + OUT],
                                Sw[:, b - b0, :], start=st, stop=sp)
                    finalize_sw(it, s, pwT, ivd)
                if it < DEPTH:
                    nc.gpsimd.collective_compute(
                        "AllGather", AO.bypass,
                        replica_groups=[list(range(NCORES))],
                        ins=[tsrc[it].opt()], outs=[tglob[it].opt()])

    nc.compile()
    return nc


_CACHE = {}


def kernel(e, p, gamma, beta, W_i, b_i, W_h, b_h, src, dst, num_nodes):
    e = np.asarray(e, np.float32)
    p = np.asarray(p, np.float32)
    src = np.asarray(src, np.int64)
    dst = np.asarray(dst, np.int64)
    N = int(num_nodes)
    OUT = int(np.asarray(W_i).shape[1])

    plan = Plan(src, dst, N)
    sig = plan.signature()
    if sig not in _CACHE:
        _CACHE[sig] = _build(plan, OUT)
    nc = _CACHE[sig]

    per_core = _host_inputs(plan, e, p, src, dst)
    wts = _weight_inputs(plan, np.asarray(gamma), np.asarray(beta),
                         np.asarray(W_i), np.asarray(b_i),
                         np.asarray(W_h), np.asarray(b_h))
    in_maps = [dict(m, **wts) for m in per_core]

    res = run_bass_kernel_spmd(nc, in_maps, core_ids=list(range(NCORES)))
    fn = np.concatenate([np.asarray(res.results[r]["out_fnT"],
                                    np.float32)[:, :plan.NLOC].T
                         for r in range(NCORES)], 0)[:N]
    h = np.concatenate([np.asarray(res.results[r]["out_hT"],
                                   np.float32)[:, :plan.NLOC].T
                        for r in range(NCORES)], 0)[:N]
    return np.concatenate([fn, h], axis=1)


# revision 23
# speedup vs baseline: 1.6414x; 1.0692x over previous
"""GCN encoder (edge-wise message passing) on 8 Trainium2 NeuronCores.

Strategy (dst-range sharding, v2):
  - Host: sort edges by dst, shard by dst-range (core r owns nodes
    [r*NLOC, (r+1)*NLOC)), group edges into 128-node windows, pad each
    (window, src-half) group to 128-multiples. Degree / index prep on host.
  - Device: BN stats via ACT-accumulate + tiny AllReduce, folded into W_i.
    Pre-pass computes f_e and the loop-invariant per-edge
    base = f_e @ Wh_mid + p*w_p (stored p-major bf16 in HBM), and performs
    the iter-0 scatter from f_e.
    Each iteration: gather g_s[src] (from the AllGathered global src-table)
    and g_d[dst] (from the local dst-table) via dma_gather,
    eh = relu(base + g_s + g_d), scatter-mean via one-hot-moving matmul
    (stationary = eh chunk) accumulating a feature-major node state
    hT [100, 512] in PSUM per superwindow; finalize scales by 1/deg and
    emits the next src/dst tables with a single matmul per window.
    Only the [NLOCP, 128] src-table is AllGathered.
  - Outputs are feature-major [100, NLOCP]; host transposes.
"""
import sys
sys.path.insert(0, "/opt/trn_rl_repo")

import numpy as np
import ml_dtypes
from contextlib import ExitStack

from concourse import bass, bacc, mybir, tile, masks
from concourse.bass_utils import run_bass_kernel_spmd

f32 = mybir.dt.float32
bf16 = mybir.dt.bfloat16
i16 = mybir.dt.int16
i32 = mybir.dt.int32
AO = mybir.AluOpType
AF = mybir.ActivationFunctionType

NCORES = 8
DEPTH = 3
EPS = 1e-5
GW = 4            # windows per superwindow
STAT_SLICE = 2048
GPIECE = 1024    # max slots per dma_gather call (SWDGE ring holds 1024 descs)

bfl = ml_dtypes.bfloat16


def _ru(x, m):
    return (x + m - 1) // m * m


class Plan:
    """Host-side preprocessing: sharding, sorting, padding, index layout."""

    def __init__(self, src, dst, N):
        E = src.shape[0]
        self.N, self.E = N, E
        self.NLOC = (N + NCORES - 1) // NCORES
        self.NWIN = (self.NLOC + 127) // 128
        self.NLOCP = self.NWIN * 128
        self.NGLOB = NCORES * self.NLOCP
        # src-half split: largest rank-multiple of NLOCP that fits int16
        self.SPLIT = min((32768 // self.NLOCP) * self.NLOCP, self.NGLOB)
        assert self.NGLOB - self.SPLIT < 32768

        owner = dst // self.NLOC
        local = dst - owner * self.NLOC
        win = local >> 7
        self.ohval_all = (local & 127).astype(np.float32)
        srcrow = (src // self.NLOC) * self.NLOCP + (src % self.NLOC)
        half = (srcrow >= self.SPLIT).astype(np.int64)
        self.srcrow, self.local, self.owner, self.win, self.half = (
            srcrow, local, owner, win, half)

        key = (owner * self.NWIN + win) * 2 + half
        self.order = np.argsort(key, kind="stable")
        cnt = np.bincount(key, minlength=NCORES * self.NWIN * 2)
        cnt = cnt.reshape(NCORES, self.NWIN, 2)
        self.capA = np.maximum(_ru(cnt[:, :, 0].max(0), 128), 128)
        self.capB = _ru(cnt[:, :, 1].max(0), 128)
        self.cnt = cnt

        # superwindows
        self.NSW = (self.NWIN + GW - 1) // GW
        self.sw_windows = [list(range(s * GW, min((s + 1) * GW, self.NWIN)))
                           for s in range(self.NSW)]
        # slot layout: per sw, [A_w0..A_wk | B_w0..B_wk]
        self.slotA = np.zeros(self.NWIN, np.int64)   # slot offset of A group
        self.slotB = np.zeros(self.NWIN, np.int64)
        self.sw_off = np.zeros(self.NSW + 1, np.int64)
        off = 0
        for s, ws in enumerate(self.sw_windows):
            self.sw_off[s] = off
            a = off
            for w in ws:
                self.slotA[w] = a
                a += self.capA[w]
                self.slotB[w] = a
                a += self.capB[w]
            off = a
        self.sw_off[self.NSW] = off
        self.ES = int(off)
        self.sw_capA = [int(sum(self.capA[w] for w in ws))
                        for ws in self.sw_windows]
        self.sw_capB = [int(sum(self.capB[w] for w in ws))
                        for ws in self.sw_windows]
        self.sw_cap = [a + b for a, b in zip(self.sw_capA, self.sw_capB)]
        self.EMAX4 = _ru(max(int((owner == r).sum()) for r in range(NCORES)), 512)
        self.Q4 = self.EMAX4 // 4

    def signature(self):
        return (self.N, self.E, tuple(self.capA), tuple(self.capB))


def _host_inputs(plan, e, p, src, dst):
    """Build the per-core input arrays."""
    NLOC, NWIN, ES = plan.NLOC, plan.NWIN, plan.ES
    order, cnt = plan.order, plan.cnt
    deg = np.maximum(np.bincount(dst, minlength=plan.N), 1).astype(np.float32)
    invd = 1.0 / deg

    in_maps = []
    pos = 0
    # order slices per (r, w, h) in key order
    slices = {}
    for r in range(NCORES):
        for w in range(NWIN):
            for h in range(2):
                c = int(cnt[r, w, h])
                slices[(r, w, h)] = order[pos:pos + c]
                pos += c
    assert pos == plan.E

    for r in range(NCORES):
        efm = np.zeros((34, ES), np.float32)
        efm[32, :] = 1.0
        gsx = np.zeros(ES, np.int16)
        gdx = np.zeros(ES, np.int16)
        ohv = np.full(ES, -5.0, np.float32)
        for w in range(NWIN):
            for h, base_slot in ((0, plan.slotA[w]), (1, plan.slotB[w])):
                idx = slices[(r, w, h)]
                n = idx.shape[0]
                sl = slice(base_slot, base_slot + n)
                efm[0:32, sl] = e[idx].T
                efm[33, sl] = p[idx, 0]
                gsx[sl] = plan.srcrow[idx] - (plan.SPLIT if h else 0)
                gdx[sl] = plan.local[idx]
                ohv[sl] = plan.ohval_all[idx]

        soh = ohv.reshape(-1, 128).T.copy()  # [128, ES//128]
        ivl = np.ones(plan.NLOCP, np.float32)
        lo, hi = r * NLOC, min((r + 1) * NLOC, plan.N)
        ivl[:hi - lo] = invd[lo:hi]
        invdb = np.tile(ivl[None, :], (128, 1))  # [128, NLOCP]

        mask = plan.owner == np.int64(r)
        er = e[mask]
        epad = np.zeros((plan.EMAX4, 32), np.float32)
        epad[:er.shape[0]] = er
        e4 = epad.reshape(4, plan.Q4, 32).transpose(0, 2, 1).reshape(128, plan.Q4)

        # gather idxs: [16, ES//16] wrapped, replicated 8x across partitions
        # (each of the 8 GPSIMD cores reads its own 16-partition copy)
        in_maps.append({
            "efm": efm.astype(bfl),
            "gs_idx": np.tile(gsx.reshape(-1, 16).T, (8, 1)),  # [128, ES//16]
            "soh": soh,
            "stf": (ohv[None, :] == np.arange(128, dtype=np.float32)[:, None]
                    ).astype(ml_dtypes.float8_e4m3),
            "invdb": invdb,
            "e4": e4.astype(bfl),
        })
    return in_maps


def _weight_inputs(plan, gamma, beta, W_i, b_i, W_h, b_h):
    OUT = W_i.shape[1]
    whmid = np.zeros((OUT + 1, 128), np.float32)
    whmid[:OUT, :OUT] = W_h[OUT:2 * OUT]
    whmid[OUT, :OUT] = W_h[2 * OUT]
    whsd = np.zeros((OUT, 256), np.float32)
    whsd[:, 0:OUT] = W_h[0:OUT]
    whsd[:, 128:128 + OUT] = W_h[2 * OUT + 1:3 * OUT + 1]
    bhb = np.zeros((128, 128), np.float32)
    bhb[:, 0:OUT] = b_h[None, :]
    return {
        "W_i": W_i.astype(np.float32),
        "b_i": b_i.reshape(OUT, 1).astype(np.float32),
        "gamma": gamma.reshape(32, 1).astype(np.float32),
        "beta": beta.reshape(32, 1).astype(np.float32),
        "whmid": whmid.astype(bfl),
        "whsd": whsd.astype(bfl),
        "bhb": bhb.astype(bfl),
    }


def _build(plan, OUT):
    """Build + compile the SPMD Bass program for this plan."""
    NWIN, NSW, ES = plan.NWIN, plan.NSW, plan.ES
    NLOCP, NGLOB, SPLIT = plan.NLOCP, plan.NGLOB, plan.SPLIT
    IN = 32

    nc = bacc.Bacc("TRN2", target_bir_lowering=False, debug=False,
                   num_devices=NCORES)

    efm = nc.dram_tensor("efm", [34, ES], bf16, kind="ExternalInput")
    gs_idx = nc.dram_tensor("gs_idx", [128, ES // 16], i16, kind="ExternalInput")
    soh = nc.dram_tensor("soh", [128, ES // 128], f32, kind="ExternalInput")
    stf = nc.dram_tensor("stf", [128, ES], mybir.dt.float8e4,
                         kind="ExternalInput")
    invdb = nc.dram_tensor("invdb", [128, NLOCP], f32, kind="ExternalInput")
    e4 = nc.dram_tensor("e4", [128, plan.Q4], bf16, kind="ExternalInput")
    W_i = nc.dram_tensor("W_i", [IN, OUT], f32, kind="ExternalInput")
    b_i = nc.dram_tensor("b_i", [OUT, 1], f32, kind="ExternalInput")
    gamma = nc.dram_tensor("gamma", [IN, 1], f32, kind="ExternalInput")
    beta = nc.dram_tensor("beta", [IN, 1], f32, kind="ExternalInput")
    whmid = nc.dram_tensor("whmid", [OUT + 1, 128], bf16,
                           kind="ExternalInput")
    whsd = nc.dram_tensor("whsd", [OUT, 256], bf16, kind="ExternalInput")
    bhb = nc.dram_tensor("bhb", [128, 128], bf16, kind="ExternalInput")

    out_fnT = nc.dram_tensor("out_fnT", [OUT, NLOCP], f32, kind="ExternalOutput")
    out_hT = nc.dram_tensor("out_hT", [OUT, NLOCP], f32, kind="ExternalOutput")

    inv_E = 1.0 / plan.E

    with tile.TileContext(nc) as tc:
        with ExitStack() as ctx:
            cpool = ctx.enter_context(tc.tile_pool(name="cpool", bufs=1))
            pool = ctx.enter_context(tc.tile_pool(name="pool", bufs=2))
            spool = ctx.enter_context(tc.tile_pool(name="spool", bufs=2))
            psum = ctx.enter_context(tc.tile_pool(name="psum", bufs=2,
                                                  space="PSUM"))
            dram = ctx.enter_context(tc.tile_pool(name="dram", bufs=1,
                                                  space="DRAM"))

            # ---- constants ----
            iota_i = cpool.tile([128, 128], i32)
            nc.gpsimd.iota(iota_i[:], pattern=[[1, 128]], base=0,
                           channel_multiplier=0)
            iota_b = cpool.tile([128, 128], bf16)
            nc.vector.tensor_copy(iota_b[:], iota_i[:])

            identf = cpool.tile([128, 128], f32)
            masks.make_identity(nc, identf[:])
            identb = cpool.tile([128, 128], bf16)
            nc.vector.tensor_copy(identb[:], identf[:])

            whmid_t = cpool.tile([OUT + 1, 128], bf16)
            nc.sync.dma_start(whmid_t[:], whmid[:])
            whsd_t = cpool.tile([OUT, 256], bf16)
            nc.sync.dma_start(whsd_t[:], whsd[:])
            bhb_t = cpool.tile([128, 128], bf16)
            nc.sync.dma_start(bhb_t[:], bhb[:])
            tblD_sb = cpool.tile([128, NWIN * 128], bf16)

            # ---- BN stats: per-core partial sums of e, e^2 ----
            nsl = (plan.Q4 + STAT_SLICE - 1) // STAT_SLICE
            parts = cpool.tile([128, 2 * nsl], f32)
            for s in range(nsl):
                c0, c1 = s * STAT_SLICE, min((s + 1) * STAT_SLICE, plan.Q4)
                esl = spool.tile([128, STAT_SLICE], bf16, tag="esl")
                nc.sync.dma_start(esl[:, :c1 - c0], e4[:, c0:c1])
                junk = spool.tile([128, STAT_SLICE], f32, tag="junk")
                nc.scalar.activation(junk[:, :c1 - c0], esl[:, :c1 - c0],
                                     AF.Copy, accum_out=parts[:, s:s + 1])
                nc.scalar.activation(junk[:, :c1 - c0], esl[:, :c1 - c0],
                                     AF.Square,
                                     accum_out=parts[:, nsl + s:nsl + s + 1])
            sums = cpool.tile([128, 2], f32)
            junk2 = cpool.tile([128, nsl], f32)
            nc.scalar.activation(junk2[:], parts[:, 0:nsl], AF.Copy,
                                 accum_out=sums[:, 0:1])
            nc.scalar.activation(junk2[:], parts[:, nsl:2 * nsl], AF.Copy,
                                 accum_out=sums[:, 1:2])
            ar_in = dram.tile([128, 2], f32)
            ar_out = dram.tile([128, 2], f32)
            nc.sync.dma_start(ar_in[:], sums[:])
            nc.gpsimd.collective_compute(
                "AllReduce", AO.add, replica_groups=[list(range(NCORES))],
                ins=[ar_in.opt()], outs=[ar_out.opt()])
            g4 = cpool.tile([32, 4, 2], f32)
            nc.sync.dma_start(
                g4[:], ar_out[:].rearrange("(g p) k -> p g k", g=4))
            t1 = cpool.tile([32, 2], f32)
            t2 = cpool.tile([32, 2], f32)
            tot = cpool.tile([32, 2], f32)
            nc.vector.tensor_tensor(t1[:], g4[:, 0, :], g4[:, 1, :], AO.add)
            nc.vector.tensor_tensor(t2[:], g4[:, 2, :], g4[:, 3, :], AO.add)
            nc.vector.tensor_tensor(tot[:], t1[:], t2[:], AO.add)
            mu = cpool.tile([32, 1], f32)
            nc.vector.tensor_scalar(mu[:], tot[:, 0:1], inv_E, None, op0=AO.mult)
            ms = cpool.tile([32, 1], f32)
            nc.vector.tensor_scalar(ms[:], tot[:, 1:2], inv_E, None, op0=AO.mult)
            var = cpool.tile([32, 1], f32)
            mu2 = cpool.tile([32, 1], f32)
            nc.vector.tensor_tensor(mu2[:], mu[:], mu[:], AO.mult)
            nc.vector.tensor_tensor(var[:], ms[:], mu2[:], AO.subtract)
            epsb = cpool.tile([32, 1], f32)
            nc.vector.memset(epsb[:], EPS)
            std = cpool.tile([32, 1], f32)
            nc.scalar.activation(std[:], var[:], AF.Sqrt, bias=epsb[:])
            rstd = cpool.tile([32, 1], f32)
            nc.vector.reciprocal(rstd[:], std[:])
            gam_t = cpool.tile([32, 1], f32)
            nc.sync.dma_start(gam_t[:], gamma[:])
            bet_t = cpool.tile([32, 1], f32)
            nc.sync.dma_start(bet_t[:], beta[:])
            a_t = cpool.tile([32, 1], f32)
            nc.vector.tensor_tensor(a_t[:], gam_t[:], rstd[:], AO.mult)
            nma = cpool.tile([32, 1], f32)
            nc.vector.scalar_tensor_tensor(nma[:], mu[:], -1.0, a_t[:],
                                           op0=AO.mult, op1=AO.mult)
            c_t = cpool.tile([32, 1], f32)
            nc.vector.tensor_tensor(c_t[:], bet_t[:], nma[:], AO.add)

            wi_t = cpool.tile([32, OUT], f32)
            nc.sync.dma_start(wi_t[:], W_i[:])
            wif = cpool.tile([32, OUT], f32)
            nc.vector.tensor_scalar(wif[:], wi_t[:], a_t[:], None, op0=AO.mult)
            bi_t = cpool.tile([OUT, 1], f32)
            nc.sync.dma_start(bi_t[:], b_i[:])
            pb = psum.tile([OUT, 1], f32, tag="ptab", bufs=1)
            nc.tensor.matmul(pb[:], wif[:], c_t[:], start=True, stop=True)
            bcol = cpool.tile([OUT, 1], f32)
            nc.vector.tensor_tensor(bcol[:], pb[:], bi_t[:], AO.add)
            scr = dram.tile([OUT, 1], f32)
            nc.sync.dma_start(scr[:], bcol[:])
            # wiaug: [33, 128] (cols 100:128 zero so fee psum is fully written)
            wiaug = cpool.tile([33, 128], bf16)
            nc.vector.memset(wiaug[:], 0.0)
            nc.vector.tensor_copy(wiaug[0:32, :OUT], wif[:])
            nc.gpsimd.dma_start(wiaug[32:33, :OUT],
                                scr[:].rearrange("a b -> b a"))

            # ---- DRAM intermediates ----
            baseH = dram.tile([128, (ES // 128) * OUT], bf16)
            tsrc = [dram.tile([NLOCP, 128], bf16, name=f"tsrc{k}",
                              tag=f"tsrc{k}") for k in range(DEPTH)]
            tglob = [dram.tile([NGLOB, 128], bf16, name=f"tglob{k}",
                               tag=f"tglob{k}") for k in range(DEPTH)]

            def sw_blocks(s):
                """Per sw-local block: (w, wl, start_flag, stop_flag)."""
                o0 = int(plan.sw_off[s])
                nblk = plan.sw_cap[s] // 128
                info = [None] * nblk
                for wl, w in enumerate(plan.sw_windows[s]):
                    blocks = []
                    for base_slot, capw in ((plan.slotA[w], plan.capA[w]),
                                            (plan.slotB[w], plan.capB[w])):
                        b0 = (int(base_slot) - o0) // 128
                        blocks += list(range(b0, b0 + int(capw) // 128))
                    for i, b in enumerate(blocks):
                        info[b] = (w, wl, i == 0, i == len(blocks) - 1)
                return info

            def build_Sw(sohc, b0, nb):
                """Edge-major one-hots for blocks [b0, b0+nb), one per block
                (DVE stride-0 broadcast APs are not supported by hardware)."""
                Sw = pool.tile([128, 4, 128], bf16, tag="S", bufs=4)
                for i in range(nb):
                    nc.vector.tensor_scalar(Sw[:, i, :], iota_b[:],
                                            sohc[:, b0 + i:b0 + i + 1], None,
                                            op0=AO.is_equal)
                return Sw

            def finalize_sw(it, s, pwT, ivd):
                """Scale by 1/deg, write outputs / next tables."""
                ws = plan.sw_windows[s]
                wcols = len(ws) * 128
                n0 = ws[0] * 128
                if it == 0 or it == DEPTH:
                    out_t = out_fnT if it == 0 else out_hT
                    hf = pool.tile([OUT, 512], f32, tag="hf")
                    nc.vector.tensor_tensor(hf[:, :wcols], pwT[0:OUT, :wcols],
                                            ivd[0:OUT, :wcols], AO.mult)
                    nc.sync.dma_start(out_t[:, n0:n0 + wcols],
                                      hf[:, :wcols])
                if it == DEPTH:
                    return
                hsc = pool.tile([OUT, 512], bf16, tag="hsc")
                nc.vector.tensor_tensor(hsc[:, :wcols], pwT[0:OUT, :wcols],
                                        ivd[0:OUT, :wcols], AO.mult)
                for wl, w in enumerate(ws):
                    ptab = psum.tile([128, 256], f32, tag="ptab", bufs=1)
                    nc.tensor.matmul(ptab[:], hsc[:, wl * 128:(wl + 1) * 128],
                                     whsd_t[:], start=True, stop=True)
                    ttab = pool.tile([128, 128], bf16, tag="ttab", bufs=4)
                    nc.scalar.copy(ttab[:], ptab[:, 0:128])
                    nc.vector.scalar_tensor_tensor(
                        tblD_sb[:, w * 128:(w + 1) * 128], ptab[:, 128:256],
                        0.0, bhb_t[:], op0=AO.add, op1=AO.add)
                    nc.sync.dma_start(tsrc[it][w * 128:(w + 1) * 128, :],
                                      ttab[:])

            # ---- pre-pass + iter 0 ----
            for s in range(NSW):
                cap = plan.sw_cap[s]
                nblk = cap // 128
                o0 = int(plan.sw_off[s])
                n0 = plan.sw_windows[s][0] * 128
                wcols = len(plan.sw_windows[s]) * 128
                binfo = sw_blocks(s)
                efm_t = pool.tile([34, cap], bf16, tag="big0")
                nc.sync.dma_start(efm_t[:], efm[:, o0:o0 + cap])
                sohc = pool.tile([128, nblk], f32, tag="sohc")
                nc.sync.dma_start(sohc[:], soh[:, o0 // 128:o0 // 128 + nblk])
                ivd = pool.tile([128, 512], f32, tag="ivd")
                nc.sync.dma_start(ivd[:, :wcols], invdb[:, n0:n0 + wcols])
                feT = pool.tile([OUT + 1, cap], bf16, tag="big1")
                baseC = pool.tile([128, nblk, OUT], bf16, tag="big2", bufs=3)

                for g0 in range(0, cap, 512):
                    g1 = min(g0 + 512, cap)
                    p1 = psum.tile([OUT, 512], f32, tag="p1", bufs=1)
                    nc.tensor.matmul(p1[:, :g1 - g0], wiaug[:, :OUT],
                                     efm_t[0:33, g0:g1], start=True, stop=True)
                    nc.scalar.activation(feT[0:OUT, g0:g1], p1[:, :g1 - g0],
                                         AF.Relu)

                nc.sync.dma_start(feT[OUT:OUT + 1, :], efm_t[33:34, :])
                pwT = psum.tile([128, 512], f32, tag="pw")
                for g0 in range(0, cap, 512):
                    g1 = min(g0 + 512, cap)
                    b0, b1 = g0 // 128, g1 // 128
                    pbs = psum.tile([128, 4, 128], f32, tag="pbase")
                    pfe = psum.tile([128, 512], f32, tag="pfee")
                    for c0 in range(g0, g1, 128):
                        sl = slice(c0, c0 + 128)
                        ci = (c0 - g0) // 128
                        cc = c0 - g0
                        nc.tensor.matmul(pbs[:, ci, :], feT[:, sl],
                                         whmid_t[:], start=True, stop=True)
                        nc.tensor.matmul(pfe[:, cc:cc + 128],
                                         efm_t[0:33, sl], wiaug[:],
                                         start=True, stop=True)
                    nc.vector.tensor_copy(baseC[:, b0:b1, :],
                                          pbs[:, 0:b1 - b0, 0:OUT])
                    fee = pool.tile([128, 512], bf16, tag="feeg")
                    nc.scalar.activation(fee[:, :g1 - g0], pfe[:, :g1 - g0],
                                         AF.Relu)
                    Sw = build_Sw(sohc, b0, b1 - b0)
                    for b in range(b0, b1):
                        w, wl, st, sp = binfo[b]
                        cc = b * 128 - g0
                        nc.tensor.matmul(pwT[0:OUT, wl * 128:(wl + 1) * 128],
                                         fee[:, cc:cc + OUT], Sw[:, b - b0, :],
                                         start=st, stop=sp)
                ob = (o0 // 128) * OUT
                nc.sync.dma_start(baseH[:, ob:ob + nblk * OUT], baseC[:])
                finalize_sw(0, s, pwT, ivd)

            nc.gpsimd.collective_compute(
                "AllGather", AO.bypass,
                replica_groups=[list(range(NCORES))],
                ins=[tsrc[0].opt()], outs=[tglob[0].opt()])

            # ---- iterations 1..DEPTH ----
            for it in range(1, DEPTH + 1):
                for s in range(NSW):
                    cap = plan.sw_cap[s]
                    nblk = cap // 128
                    capA = plan.sw_capA[s]
                    o0 = int(plan.sw_off[s])
                    n0 = plan.sw_windows[s][0] * 128
                    wcols = len(plan.sw_windows[s]) * 128
                    binfo = sw_blocks(s)
                    GS = pool.tile([128, nblk, 128], bf16, tag="big1")
                    BASE = pool.tile([128, nblk, OUT], bf16, tag="big2",
                                     bufs=3)
                    ob = (o0 // 128) * OUT
                    nc.sync.dma_start(BASE[:], baseH[:, ob:ob + nblk * OUT])
                    sohc = pool.tile([128, nblk], f32, tag="sohc")
                    nc.sync.dma_start(sohc[:],
                                      soh[:, o0 // 128:o0 // 128 + nblk])
                    ivd = pool.tile([128, 512], f32, tag="ivd")
                    nc.sync.dma_start(ivd[:, :wcols], invdb[:, n0:n0 + wcols])
                    STt = pool.tile([128, cap], mybir.dt.float8e4, tag="STt")
                    nc.sync.dma_start(STt[:], stf[:, o0:o0 + cap])
                    gsix = pool.tile([128, cap // 16], i16, tag="gsix")
                    nc.sync.dma_start(gsix[:],
                                      gs_idx[:, o0 // 16:(o0 + cap) // 16])

                    # pieces of <=GPIECE slots within each (window, half)
                    # region run (layout is window-major: A_w B_w A_w+1 ...)
                    pieces = []
                    for w in plan.sw_windows[s]:
                        for base_slot, capw, isA in (
                                (plan.slotA[w], plan.capA[w], True),
                                (plan.slotB[w], plan.capB[w], False)):
                            q = int(base_slot) - o0
                            r1 = q + int(capw)
                            while q < r1:
                                m = min(GPIECE, r1 - q)
                                pieces.append((q, m, isA))
                                q += m

                    for (q, m, isA) in pieces:
                        b0 = q // 128
                        src_v = (tglob[it - 1][:] if isA
                                 else tglob[it - 1][SPLIT:, :])
                        nc.gpsimd.dma_gather(
                            GS[:, b0:b0 + m // 128, :], src_v,
                            gsix[:, q // 16:(q + m) // 16], m, m, 128)

                    GSf = GS[:].rearrange("p b c -> p (b c)")
                    pwT = psum.tile([128, 512], f32, tag="pw")
                    for g0 in range(0, cap, 512):
                        g1 = min(g0 + 512, cap)
                        gcols = g1 - g0
                        b0, b1 = g0 // 128, g1 // 128
                        nc.vector.scalar_tensor_tensor(
                            GS[:, b0:b1, 0:OUT], GS[:, b0:b1, 0:OUT], 0.0,
                            BASE[:, b0:b1, :], op0=AO.add, op1=AO.add)
                        u = psum.tile([128, 512], f32, tag="pbase")
                        nc.tensor.matmul(u[:, :gcols], identb[:],
                                         GSf[:, g0:g1], start=True, stop=True)
                        for b in range(b0, b1):
                            w = binfo[b][0]
                            cc = b * 128 - g0
                            nc.tensor.matmul(
                                u[:, cc:cc + 128], STt[:, b * 128:b * 128 + 128],
                                tblD_sb[:, w * 128:(w + 1) * 128],
                                start=False, stop=True, skip_group_check=True)
                        nc.scalar.activation(GSf[:, g0:g1], u[:, :gcols],
                                             AF.Relu)
                        Sw = build_Sw(sohc, b0, b1 - b0)
                        for b in range(b0, b1):
                            w, wl, st, sp = binfo[b]
                            nc.tensor.matmul(
                                pwT[0:OUT, wl * 128:(wl + 1) * 128],
                                GSf[:, b * 128:b * 128 + OUT],
                                Sw[:, b - b0, :], start=st, stop=sp)
                    finalize_sw(it, s, pwT, ivd)
                if it < DEPTH:
                    nc.gpsimd.collective_compute(
                        "AllGather", AO.bypass,
                        replica_groups=[list(range(NCORES))],
                        ins=[tsrc[it].opt()], outs=[tglob[it].opt()])

    nc.compile()
    return nc


_CACHE = {}


def kernel(e, p, gamma, beta, W_i, b_i, W_h, b_h, src, dst, num_nodes):
    e = np.asarray(e, np.float32)
    p = np.asarray(p, np.float32)
    src = np.asarray(src, np.int64)
    dst = np.asarray(dst, np.int64)
    N = int(num_nodes)
    OUT = int(np.asarray(W_i).shape[1])

    plan = Plan(src, dst, N)
    sig = plan.signature()
    if sig not in _CACHE:
        _CACHE[sig] = _build(plan, OUT)
    nc = _CACHE[sig]

    per_core = _host_inputs(plan, e, p, src, dst)
    wts = _weight_inputs(plan, np.asarray(gamma), np.asarray(beta),
                         np.asarray(W_i), np.asarray(b_i),
                         np.asarray(W_h), np.asarray(b_h))
    in_maps = [dict(m, **wts) for m in per_core]

    res = run_bass_kernel_spmd(nc, in_maps, core_ids=list(range(NCORES)))
    fn = np.concatenate([np.asarray(res.results[r]["out_fnT"],
                                    np.float32)[:, :plan.NLOC].T
                         for r in range(NCORES)], 0)[:N]
    h = np.concatenate([np.asarray(res.results[r]["out_hT"],
                                   np.float32)[:, :plan.NLOC].T
                        for r in range(NCORES)], 0)[:N]
    return np.concatenate([fn, h], axis=1)
